# revision 1
# baseline (speedup 1.0000x reference)
"""Trainium2 Bass kernel for a 3-layer minLSTM-style NLP model.

Model (per reference):
  x = emb[ids]                                   (B,S,E) = (2,2048,512)
  3 x { xn = LN(x); gates = xn @ Ws.T + bs;
        f' = sig(sp(-i)-sp(-f)); i' = sig(sp(-f)-sp(-i));
        v = i' * g(tilde), g(x) = max(x+0.5, sigmoid(x));
        h_t = f'_t h_{t-1} + v_t  (h_0 = 0.5);  x = h + x }
  xf = LN(x) * fln_w;  logits = xf @ fc_w.T + fc_b    (B,S,32000)

Sharding (8 cores, zero collectives):
  core c -> (batch b=c//4, seq chunk j=c%4 of 512 tokens). Each core runs a
  768-token window (256-token halo before its own 512) through the recurrent
  stack; the forget-product decays the unknown initial state to ~1e-10 over
  the halo, and a per-core reset constant makes j==0 exact at the batch start.
  Each core then computes logits for its own 512 tokens against the full
  vocab (fc_w pre-transposed/tiled/bf16-cast on host) and writes its
  [512, 32000] f32 shard; the host concatenates shards.

Layout: activations channel-major [H on partitions x tokens free]; LN stats
via PE partition-reduction; LN affine via rank-1 PE broadcasts; recurrence
via the DVE tensor_tensor_scan instruction.
"""

import sys

if "/opt/trn_rl_repo" not in sys.path:
    sys.path.insert(0, "/opt/trn_rl_repo")

import numpy as np

import concourse.bass as bass
import concourse.bacc as bacc
import concourse.tile as tile
from concourse import mybir
from concourse.bass import IndirectOffsetOnAxis
from concourse.bass_utils import run_bass_kernel_spmd
from concourse.masks import make_identity

F32 = mybir.dt.float32
F16 = mybir.dt.float16
I32 = mybir.dt.int32
AF = mybir.ActivationFunctionType
OP = mybir.AluOpType

# problem constants
B, S, V, H, L = 2, 2048, 32000, 512, 3
P = 128
KT = H // P            # 4 k-tiles over the H contraction dim
CHUNK = 512            # own tokens per core
HALO = 128             # speculative scan warmup tokens
W = HALO + CHUNK       # 768 window tokens per core
NG = W // P            # 6 embedding gather groups
NCH = [(0, 512), (512, 128)]   # window free-dim chunks (PSUM-bounded)
OWN = (HALO, CHUNK)            # own-token slice of window
VC = 500               # vocab chunk for logits
NV = V // VC           # 64 vocab chunks
N_CORES = 8
EPS = 1e-5


def _bcast_row(ap_1d, off, n, p=P):
    """[n] slice of a 1-D DRAM tensor broadcast across p partitions."""
    return bass.AP(tensor=ap_1d.tensor, offset=ap_1d.offset + off,
                   ap=[[0, p], [1, n]])


def build_program(skip_phase1=False, skip_phasec=False, workb=2, statb=1, wstb=1, psgb=3, psbb=1, fcwb=9, ldsync=False):
    nc = bacc.Bacc("TRN2", target_bir_lowering=False, debug=False,
                   enable_asserts=True, num_devices=N_CORES)

    idx_t = nc.dram_tensor("idx", [P, NG], I32, kind="ExternalInput").ap()
    emb_t = nc.dram_tensor("emb", [V, H], F32, kind="ExternalInput").ap()
    wsT_t = nc.dram_tensor("wsT", [L, KT, P, 3 * H], F16, kind="ExternalInput").ap()
    bsg_t = nc.dram_tensor("bsg", [P, L * 12], F32, kind="ExternalInput").ap()
    bshalf_t = nc.dram_tensor("bshalf", [P, L * 4], F32, kind="ExternalInput").ap()
    lnb1_t = nc.dram_tensor("lnb1", [1, (L + 1) * H], F16, kind="ExternalInput").ap()
    lnw1_t = nc.dram_tensor("lnw1", [1, (L + 1) * H], F16, kind="ExternalInput").ap()
    fcwt_t = nc.dram_tensor("fcwt", [25, P, 10, KT, P], F16, kind="ExternalInput").ap()
    fcb_t = nc.dram_tensor("fcb", [P, V // P], F32, kind="ExternalInput").ap()
    rst_t = nc.dram_tensor("rst", [P, 2], F32, kind="ExternalInput").ap()
    out_t = nc.dram_tensor("out", [V, CHUNK], F32, kind="ExternalOutput").ap()

    with tile.TileContext(nc) as tc:
        with tc.tile_pool(name="singles", bufs=1) as singles, \
             tc.tile_pool(name="persist", bufs=1) as persist, \
             tc.tile_pool(name="fcw", bufs=fcwb) as fcwp:

            # ---- constants / small inputs ----
            idx = singles.tile([P, NG], I32)
            nc.sync.dma_start(out=idx[:], in_=idx_t[:])
            bsg = singles.tile([P, L * 12], F32)
            nc.sync.dma_start(out=bsg[:], in_=bsg_t[:])
            bshalf = singles.tile([P, L * 4], F32)
            nc.sync.dma_start(out=bshalf[:], in_=bshalf_t[:])
            lnb1 = singles.tile([1, (L + 1) * H], F16)
            nc.sync.dma_start(out=lnb1[:], in_=lnb1_t[:])
            lnw1 = singles.tile([1, (L + 1) * H], F16)
            nc.sync.dma_start(out=lnw1[:], in_=lnw1_t[:])
            rst = singles.tile([P, 2], F32)
            nc.sync.dma_start(out=rst[:], in_=rst_t[:])
            ident = singles.tile([P, P], F32)
            make_identity(nc, ident[:])
            ones16 = singles.tile([P, 1], F16)   # stats-reduce lhsT
            nc.vector.memset(ones16[:], 1.0)
            onesrow = singles.tile([1, W], F16)   # B0 bias-broadcast rhs
            nc.vector.memset(onesrow[:], 1.0)
            eps_t = singles.tile([1, 1], F32)
            nc.vector.memset(eps_t[:], EPS)
            fcb2 = singles.tile([P, V // P], F32)
            nc.sync.dma_start(out=fcb2[:], in_=fcb_t[:])

            # x tiles (channel-major activations), rotate per layer
            xf_bf = [persist.tile([P, CHUNK], F16, tag=f"xfbf{k}", name=f"xfbf{k}")
                     for k in range(KT)]

            with tc.tile_pool(name="xpool", bufs=2) as xpool, \
                 tc.tile_pool(name="wst", bufs=wstb) as wstp, \
                 tc.tile_pool(name="work", bufs=workb) as work, \
                 tc.tile_pool(name="scan", bufs=1) as scanp, \
                 tc.tile_pool(name="stat", bufs=statb) as statp, \
                 tc.tile_pool(name="xnp", bufs=1) as xnp, \
                 tc.tile_pool(name="psg", bufs=psgb, space="PSUM") as psg, \
                 tc.tile_pool(name="pss", bufs=1, space="PSUM") as pss, \
                 tc.tile_pool(name="psb", bufs=psbb, space="PSUM") as psb:

                # ---- phase A: embedding gather + transpose to channel-major
                x = [xpool.tile([P, W], F16, tag=f"x{k}", name=f"xt{k}") for k in range(KT)]
                for g in range(NG):
                    xg = work.tile([P, H], F32, tag="xg")
                    nc.gpsimd.indirect_dma_start(
                        out=xg[:], out_offset=None, in_=emb_t[:],
                        in_offset=IndirectOffsetOnAxis(ap=idx[:, g:g + 1], axis=0),
                    )
                    for k in range(KT):
                        ptr = psg.tile([P, 512], F32, tag="pg", name="ptr")
                        nc.tensor.transpose(
                            out=ptr[:, :P], in_=xg[:, k * P:(k + 1) * P],
                            identity=ident[:])
                        nc.scalar.copy(out=x[k][:, g * P:(g + 1) * P], in_=ptr[:, :P])

                # ---- phase B: L recurrent layers ----
                for l in range(L if not skip_phase1 else 0):
                    wst = wstp.tile([P, KT * 3 * H], F16, tag="wst")
                    for kk in range(KT):
                        nc.sync.dma_start(
                            out=wst[:, kk * 3 * H:(kk + 1) * 3 * H],
                            in_=wsT_t[l, kk])

                    # --- LayerNorm (stats over channels = partition dim) ---
                    mneg = statp.tile([1, W], F32, tag="mneg")
                    ex2 = statp.tile([1, W], F32, tag="ex2")
                    rstd = statp.tile([1, W], F32, tag="rstd")
                    rstd16 = statp.tile([1, W], F16, tag="rstd16")
                    mr16 = statp.tile([1, W], F16, tag="mr16")
                    scr = statp.tile([1, W], F32, tag="scr")
                    scr2 = statp.tile([1, W], F32, tag="scr2")
                    nrt = statp.tile([1, W], F32, tag="nrt")
                    xn = [xnp.tile([P, W], F16, tag=f"xn{k}", name=f"xn{k}") for k in range(KT)]

                    for (o, n) in NCH:
                        ps_a = pss.tile([1, 512], F32, tag="ps_a")
                        ps_b = pss.tile([1, 512], F32, tag="ps_b")
                        for k in range(KT):
                            sq = work.tile([P, 512], F16, tag="sq")
                            nc.scalar.square(out=sq[:, :n], in_=x[k][:, o:o + n])
                            nc.tensor.matmul(
                                out=ps_a[:, :n], lhsT=(ones16[:]),
                                rhs=(x[k][:, o:o + n]),
                                start=(k == 0), stop=(k == KT - 1))
                            nc.tensor.matmul(
                                out=ps_b[:, :n], lhsT=(ones16[:]),
                                rhs=(sq[:, :n]),
                                start=(k == 0), stop=(k == KT - 1))
                        nc.scalar.mul(out=mneg[0:1, o:o + n], in_=ps_a[0:1, :n],
                                      mul=-1.0 / H)
                        nc.scalar.mul(out=ex2[0:1, o:o + n], in_=ps_b[0:1, :n],
                                      mul=1.0 / H)
                        # var = E[x^2] - mean^2 ; se = sqrt(var+eps); rstd = 1/se
                        nc.vector.tensor_mul(scr[0:1, o:o + n], mneg[0:1, o:o + n],
                                             mneg[0:1, o:o + n])
                        nc.vector.tensor_sub(scr[0:1, o:o + n], ex2[0:1, o:o + n],
                                             scr[0:1, o:o + n])
                    # merged rstd = 1/sqrt(var+eps) chain over the full window
                    sl = slice(0, W)
                    nc.vector.tensor_scalar_add(scr[0:1, sl], scr[0:1, sl], EPS)
                    nc.vector.tensor_scalar_mul(scr2[0:1, sl], scr[0:1, sl],
                                                -0.5)
                    nc.vector.tensor_scalar(
                        out=rstd[0:1, sl].bitcast(I32),
                        in0=scr[0:1, sl].bitcast(I32),
                        scalar1=1, scalar2=None,
                        op0=OP.arith_shift_right)
                    nc.vector.tensor_scalar(
                        out=rstd[0:1, sl].bitcast(I32),
                        in0=rstd[0:1, sl].bitcast(I32),
                        scalar1=0x5F3759DF, scalar2=None,
                        op0=OP.subtract)
                    nc.vector.tensor_scalar_mul(rstd[0:1, sl].bitcast(I32),
                                                rstd[0:1, sl].bitcast(I32), -1)
                    for _ in range(2):
                        nc.vector.tensor_mul(nrt[0:1, sl], rstd[0:1, sl],
                                             rstd[0:1, sl])
                        nc.vector.tensor_mul(nrt[0:1, sl], nrt[0:1, sl],
                                             scr2[0:1, sl])
                        nc.vector.scalar_tensor_tensor(
                            out=rstd[0:1, sl], in0=nrt[0:1, sl], scalar=1.5,
                            in1=rstd[0:1, sl], op0=OP.add, op1=OP.mult)
                    nc.scalar.copy(out=rstd16[0:1, sl], in_=rstd[0:1, sl])
                    nc.vector.tensor_mul(scr[0:1, sl], mneg[0:1, sl],
                                         rstd[0:1, sl])
                    nc.scalar.copy(out=mr16[0:1, sl], in_=scr[0:1, sl])
                    # xn = x * (lnw x rstd) + (lnb x 1 + lnw x (-m*rstd))
                    for (o, n) in NCH:
                        for k in range(KT):
                            lsl = slice(l * H + k * P, l * H + (k + 1) * P)
                            b1 = psb.tile([P, 512], F32, tag="b1")
                            nc.tensor.matmul(out=b1[:, :n],
                                             lhsT=(lnw1[0:1, lsl]),
                                             rhs=(rstd16[0:1, o:o + n]),
                                             start=True, stop=True)
                            b0 = psb.tile([P, 512], F32, tag="b0")
                            nc.tensor.matmul(out=b0[:, :n],
                                             lhsT=(lnb1[0:1, lsl]),
                                             rhs=(onesrow[0:1, o:o + n]),
                                             start=True, stop=False)
                            nc.tensor.matmul(out=b0[:, :n],
                                             lhsT=(lnw1[0:1, lsl]),
                                             rhs=(mr16[0:1, o:o + n]),
                                             start=False, stop=True)
                            nc.vector.tensor_mul(xn[k][:, o:o + n], b1[:, :n],
                                                 x[k][:, o:o + n])
                            nc.vector.tensor_add(xn[k][:, o:o + n], b0[:, :n],
                                                 xn[k][:, o:o + n])

                    # --- gates GEMM + nonlinearities + scan ---
                    fp = [scanp.tile([P, W], F16, tag=f"fp{k}", name=f"fp{k}") for k in range(KT)]
                    vv = [scanp.tile([P, W], F16, tag=f"vv{k}", name=f"vv{k}") for k in range(KT)]
                    hh = [scanp.tile([P, W], F16, tag=f"h{k}", name=f"h{k}") for k in range(KT)]
                    x2 = [xpool.tile([P, W], F16, tag=f"x{k}", name=f"xt{k}") for k in range(KT)]
                    for (o, n) in NCH:
                        for k in range(KT):
                            def gate_mm(gate):
                                pg = psg.tile([P, 512], F32, tag="pg")
                                for kk in range(KT):
                                    c0 = kk * 3 * H + gate * H + k * P
                                    nc.tensor.matmul(
                                        out=pg[:, :n],
                                        lhsT=(wst[:, c0:c0 + P]),
                                        rhs=(xn[kk][:, o:o + n]),
                                        start=(kk == 0), stop=(kk == KT - 1))
                                return pg

                            # f' = sig(f)/(sig(f)+sig(i)), v = sig(i)*g/(..)
                            pg_f = gate_mm(0)
                            sf = work.tile([P, 512], F32, tag="sf")
                            nc.scalar.activation(
                                out=sf[:, :n], in_=pg_f[:, :n], func=AF.Sigmoid,
                                bias=bsg[:, l * 12 + k:l * 12 + k + 1])
                            pg_i = gate_mm(1)
                            si = work.tile([P, 512], F32, tag="si")
                            nc.scalar.activation(
                                out=si[:, :n], in_=pg_i[:, :n], func=AF.Sigmoid,
                                bias=bsg[:, l * 12 + 4 + k:l * 12 + 4 + k + 1])
                            ssum = work.tile([P, 512], F32, tag="ssum")
                            nc.gpsimd.tensor_add(ssum[:, :n], sf[:, :n], si[:, :n])
                            rinv = work.tile([P, 512], F32, tag="rinv")
                            nc.vector.reciprocal_approx_fast(
                                out=rinv[:, :n], in_=ssum[:, :n])
                            nc.gpsimd.tensor_mul(fp[k][:, o:o + n], sf[:, :n],
                                                 rinv[:, :n])
                            pg_t = gate_mm(2)
                            sg = work.tile([P, 512], F32, tag="sg")
                            nc.scalar.activation(
                                out=sg[:, :n], in_=pg_t[:, :n], func=AF.Sigmoid,
                                bias=bsg[:, l * 12 + 8 + k:l * 12 + 8 + k + 1])
                            g = work.tile([P, 512], F32, tag="g")
                            nc.vector.scalar_tensor_tensor(
                                out=g[:, :n], in0=pg_t[:, :n],
                                scalar=bshalf[:, l * 4 + k:l * 4 + k + 1],
                                in1=sg[:, :n], op0=OP.add, op1=OP.max)
                            num = work.tile([P, 512], F32, tag="num")
                            nc.vector.tensor_mul(num[:, :n], si[:, :n], g[:, :n])
                            nc.vector.tensor_mul(vv[k][:, o:o + n], num[:, :n],
                                                 rinv[:, :n])
                            if o == 0:
                                # boundary reset at own-region start (exact
                                # for j==0 cores)
                                t1 = work.tile([P, 1], F32, tag="t1")
                                nc.vector.tensor_mul(
                                    t1[:], fp[k][:, HALO:HALO + 1], rst[:, 1:2])
                                nc.vector.tensor_add(
                                    vv[k][:, HALO:HALO + 1], t1[:],
                                    vv[k][:, HALO:HALO + 1])
                                nc.vector.tensor_mul(
                                    fp[k][:, HALO:HALO + 1],
                                    fp[k][:, HALO:HALO + 1], rst[:, 0:1])
                            else:
                                nc.vector.tensor_tensor_scan(
                                    out=hh[k][:], data0=fp[k][:],
                                    data1=vv[k][:],
                                    initial=0.5, op0=OP.mult, op1=OP.add)
                                nc.vector.tensor_add(x2[k][:], hh[k][:],
                                                     x[k][:])
                    x = x2

                # ---- final LayerNorm (own tokens only) + bf16 cast ----
                if skip_phase1:
                    for k in range(KT):
                        nc.vector.memset(xf_bf[k][:], 0.01)
                o, n = OWN
                if skip_phase1:
                    n = 0  # handled via memset above
                el_ln = not skip_phase1

                if el_ln:
                  mneg = statp.tile([1, 512], F32, tag="mneg")
                  ex2 = statp.tile([1, 512], F32, tag="ex2")
                  rstd = statp.tile([1, 512], F32, tag="rstd")
                  rstd16 = statp.tile([1, 512], F16, tag="rstd16")
                  mr16 = statp.tile([1, 512], F16, tag="mr16")
                  scr = statp.tile([1, 512], F32, tag="scr")
                  scr2 = statp.tile([1, 512], F32, tag="scr2")
                  nrt = statp.tile([1, 512], F32, tag="nrt")
                  ps_a = pss.tile([1, 512], F32, tag="ps_a")
                  ps_b = pss.tile([1, 512], F32, tag="ps_b")
                  for k in range(KT):
                      sq = work.tile([P, 512], F16, tag="sq")
                      nc.scalar.square(out=sq[:, :n], in_=x[k][:, o:o + n])
                      nc.tensor.matmul(out=ps_a[:, :n], lhsT=(ones16[:]),
                                       rhs=(x[k][:, o:o + n]),
                                       start=(k == 0), stop=(k == KT - 1))
                      nc.tensor.matmul(out=ps_b[:, :n], lhsT=(ones16[:]),
                                       rhs=(sq[:, :n]),
                                       start=(k == 0), stop=(k == KT - 1))
                  nc.scalar.mul(out=mneg[0:1, :n], in_=ps_a[0:1, :n], mul=-1.0 / H)
                  nc.scalar.mul(out=ex2[0:1, :n], in_=ps_b[0:1, :n], mul=1.0 / H)
                  nc.vector.tensor_mul(scr[0:1, :n], mneg[0:1, :n], mneg[0:1, :n])
                  nc.vector.tensor_sub(scr[0:1, :n], ex2[0:1, :n], scr[0:1, :n])
                  nc.vector.tensor_scalar_add(scr[0:1, :n], scr[0:1, :n], EPS)
                  nc.vector.tensor_scalar_mul(scr2[0:1, :n], scr[0:1, :n], -0.5)
                  nc.vector.tensor_scalar(
                      out=rstd[0:1, :n].bitcast(I32),
                      in0=scr[0:1, :n].bitcast(I32),
                      scalar1=1, scalar2=None,
                      op0=OP.arith_shift_right)
                  nc.vector.tensor_scalar(
                      out=rstd[0:1, :n].bitcast(I32),
                      in0=rstd[0:1, :n].bitcast(I32),
                      scalar1=0x5F3759DF, scalar2=None,
                      op0=OP.subtract)
                  nc.vector.tensor_scalar_mul(rstd[0:1, :n].bitcast(I32),
                                              rstd[0:1, :n].bitcast(I32), -1)
                  for _ in range(2):
                      nc.vector.tensor_mul(nrt[0:1, :n], rstd[0:1, :n], rstd[0:1, :n])
                      nc.vector.tensor_mul(nrt[0:1, :n], nrt[0:1, :n], scr2[0:1, :n])
                      nc.vector.scalar_tensor_tensor(
                          out=rstd[0:1, :n], in0=nrt[0:1, :n], scalar=1.5,
                          in1=rstd[0:1, :n], op0=OP.add, op1=OP.mult)
                  nc.scalar.copy(out=rstd16[0:1, :n], in_=rstd[0:1, :n])
                  nc.vector.tensor_mul(scr[0:1, :n], mneg[0:1, :n], rstd[0:1, :n])
                  nc.scalar.copy(out=mr16[0:1, :n], in_=scr[0:1, :n])
                  for k in range(KT):
                      lsl = slice(L * H + k * P, L * H + (k + 1) * P)
                      b1 = psb.tile([P, 512], F32, tag="b1")
                      nc.tensor.matmul(out=b1[:, :n], lhsT=(lnw1[0:1, lsl]),
                                       rhs=(rstd16[0:1, :n]), start=True, stop=True)
                      b0 = psb.tile([P, 512], F32, tag="b0")
                      nc.tensor.matmul(out=b0[:, :n], lhsT=(lnb1[0:1, lsl]),
                                       rhs=(onesrow[0:1, :n]), start=True, stop=False)
                      nc.tensor.matmul(out=b0[:, :n], lhsT=(lnw1[0:1, lsl]),
                                       rhs=(mr16[0:1, :n]), start=False, stop=True)
                      xf = work.tile([P, 512], F32, tag="xf")
                      nc.vector.tensor_mul(xf[:, :n], b1[:, :n], x[k][:, o:o + n])
                      nc.vector.tensor_add(xf_bf[k][:], b0[:, :n], xf[:, :n])

            # ---- phase C: logits GEMM (own 512 tokens x full vocab) ----
            if skip_phasec:
                for k in range(KT):
                    nc.sync.dma_start(out=out_t[k * P:(k + 1) * P, 0:256],
                                      in_=xf_bf[k][:].bitcast(F32))
            VG = 10   # vocab tiles per fcw load (25 groups of 10)
            with tc.tile_pool(name="osb", bufs=8) as osbp, \
                 tc.tile_pool(name="pso", bufs=8, space="PSUM") as pso:
                for vg in range(25 if not skip_phasec else 0):
                    fcw = fcwp.tile([P, VG, KT, P], F16, tag="fcw")
                    (nc.sync if ldsync else nc.gpsimd).dma_start(out=fcw[:], in_=fcwt_t[vg])
                    for j in range(VG):
                        vt = vg * VG + j
                        po = pso.tile([P, CHUNK], F32, tag="po")
                        for k in range(KT):
                            nc.tensor.matmul(
                                out=po[:], lhsT=fcw[:, j, k, :],
                                rhs=xf_bf[k][:],
                                start=(k == 0), stop=(k == KT - 1))
                        osb = osbp.tile([P, CHUNK], F32, tag="osb")
                        nc.scalar.activation(out=osb[:], in_=po[:],
                                             func=AF.Identity,
                                             bias=fcb2[:, vt:vt + 1])
                        nc.sync.dma_start(out=out_t[vt * P:(vt + 1) * P, :],
                                          in_=osb[:])

    nc.compile()
    return nc


_CACHED = None


def _get_program():
    global _CACHED
    if _CACHED is None:
        _CACHED = build_program()
    return _CACHED


def prep_inputs(ids, emb, Ws, bs, ln_w, ln_b, fln_w, fc_w, fc_b):
    """Host-side layout prep -> per-core input maps."""
    ids = np.asarray(ids)
    emb = np.ascontiguousarray(np.asarray(emb, dtype=np.float32))
    Ws = np.asarray(Ws, dtype=np.float32)
    bs = np.asarray(bs, dtype=np.float32)
    ln_w = np.asarray(ln_w, dtype=np.float32)
    ln_b = np.asarray(ln_b, dtype=np.float32)
    fln_w = np.asarray(fln_w, dtype=np.float32)
    fc_w = np.asarray(fc_w, dtype=np.float32)
    fc_b = np.asarray(fc_b, dtype=np.float32)

    # Ws[l].T tiled into [KT, 128, 3H], fp16
    wsT = np.ascontiguousarray(
        np.stack([Ws[l].T.reshape(KT, P, 3 * H) for l in range(L)])).astype(
            np.float16)

    # per-partition gate biases, grouped [l][gate][k]
    bsg = np.empty((P, L * 12), np.float32)
    bshalf = np.empty((P, L * 4), np.float32)
    for l in range(L):
        for gate in range(3):
            for k in range(KT):
                bsg[:, l * 12 + gate * 4 + k] =                     bs[l, gate * H + k * P:gate * H + (k + 1) * P]
        for k in range(KT):
            bshalf[:, l * 4 + k] = bs[l, 2 * H + k * P:2 * H + (k + 1) * P] + 0.5

    lnb1 = np.zeros((1, (L + 1) * H), np.float32)
    lnw1 = np.zeros((1, (L + 1) * H), np.float32)
    for l in range(L):
        lnb1[0, l * H:(l + 1) * H] = ln_b[l]
        lnw1[0, l * H:(l + 1) * H] = ln_w[l]
    lnw1[0, L * H:] = fln_w

    # fc_w.T tiled [25, 128, 10, KT, 128] fp16, vocab-major logits layout
    fcwt = np.ascontiguousarray(
        fc_w.T.reshape(KT, P, 25, 10, P).transpose(2, 1, 3, 0, 4)).astype(
            np.float16)
    fcb2 = np.ascontiguousarray(fc_b.reshape(V // P, P).T)

    shared = {"emb": emb, "wsT": wsT, "bsg": bsg,
              "bshalf": bshalf, "lnb1": lnb1.astype(np.float16),
              "lnw1": lnw1.astype(np.float16),
              "fcwt": fcwt, "fcb": fcb2}

    in_maps = []
    for c in range(N_CORES):
        b, j = divmod(c, 4)
        own0 = j * CHUNK
        win = np.zeros(W, np.int32)
        if j == 0:
            win[HALO:] = ids[b, :CHUNK]
        else:
            win[:] = ids[b, own0 - HALO:own0 + CHUNK]
        idxt = np.ascontiguousarray(win.reshape(NG, P).T)
        rstc = np.empty((P, 2), np.float32)
        rstc[:, 0] = 0.0 if j == 0 else 1.0   # multiplies f at window pos HALO
        rstc[:, 1] = 0.5 if j == 0 else 0.0   # adds f*this to v at pos HALO
        in_maps.append({**shared, "idx": idxt, "rst": rstc})
    return in_maps


def kernel(ids, emb, Ws, bs, ln_w, ln_b, fln_w, fc_w, fc_b):
    nc = _get_program()
    in_maps = prep_inputs(ids, emb, Ws, bs, ln_w, ln_b, fln_w, fc_w, fc_b)
    res = run_bass_kernel_spmd(nc, in_maps, list(range(N_CORES)))
    out = np.empty((B, S, V), np.float32)
    for c in range(N_CORES):
        b, j = divmod(c, 4)
        out[b, j * CHUNK:(j + 1) * CHUNK, :] = res.results[c]["out"].T
    return out



# revision 5
# speedup vs baseline: 1.0807x; 1.0807x over previous
"""Trainium2 Bass kernel for a 3-layer minLSTM-style NLP model.

Model (per reference):
  x = emb[ids]                                   (B,S,E) = (2,2048,512)
  3 x { xn = LN(x); gates = xn @ Ws.T + bs;
        f' = sig(sp(-i)-sp(-f)); i' = sig(sp(-f)-sp(-i));
        v = i' * g(tilde), g(x) = max(x+0.5, sigmoid(x));
        h_t = f'_t h_{t-1} + v_t  (h_0 = 0.5);  x = h + x }
  xf = LN(x) * fln_w;  logits = xf @ fc_w.T + fc_b    (B,S,32000)

Sharding (8 cores, zero collectives):
  core c -> (batch b=c//4, seq chunk j=c%4 of 512 tokens). Each core runs a
  768-token window (256-token halo before its own 512) through the recurrent
  stack; the forget-product decays the unknown initial state to ~1e-10 over
  the halo, and a per-core reset constant makes j==0 exact at the batch start.
  Each core then computes logits for its own 512 tokens against the full
  vocab (fc_w pre-transposed/tiled/bf16-cast on host) and writes its
  [512, 32000] f32 shard; the host concatenates shards.

Layout: activations channel-major [H on partitions x tokens free]; LN stats
via PE partition-reduction; LN affine via rank-1 PE broadcasts; recurrence
via the DVE tensor_tensor_scan instruction.
"""

import sys

if "/opt/trn_rl_repo" not in sys.path:
    sys.path.insert(0, "/opt/trn_rl_repo")

import numpy as np

import concourse.bass as bass
import concourse.bacc as bacc
import concourse.tile as tile
from concourse import mybir
from concourse.bass import IndirectOffsetOnAxis
from concourse.bass_utils import run_bass_kernel_spmd
from concourse.masks import make_identity

F32 = mybir.dt.float32
F16 = mybir.dt.float16
I32 = mybir.dt.int32
AF = mybir.ActivationFunctionType
OP = mybir.AluOpType

# problem constants
B, S, V, H, L = 2, 2048, 32000, 512, 3
P = 128
KT = H // P            # 4 k-tiles over the H contraction dim
CHUNK = 512            # own tokens per core
HALO = 128             # speculative scan warmup tokens
W = HALO + CHUNK       # 768 window tokens per core
NG = W // P            # 6 embedding gather groups
NCH = [(0, 512), (512, 128)]   # window free-dim chunks (PSUM-bounded)
OWN = (HALO, CHUNK)            # own-token slice of window
VC = 500               # vocab chunk for logits
NV = V // VC           # 64 vocab chunks
N_CORES = 8
EPS = 1e-5


def _bcast_row(ap_1d, off, n, p=P):
    """[n] slice of a 1-D DRAM tensor broadcast across p partitions."""
    return bass.AP(tensor=ap_1d.tensor, offset=ap_1d.offset + off,
                   ap=[[0, p], [1, n]])


def build_program(skip_phase1=False, skip_phasec=False, workb=2, statb=1, wstb=1, psgb=3, psbb=1, fcwb=9, ldsync=False):
    nc = bacc.Bacc("TRN2", target_bir_lowering=False, debug=False,
                   enable_asserts=True, num_devices=N_CORES)

    idx_t = nc.dram_tensor("idx", [P, NG], I32, kind="ExternalInput").ap()
    emb_t = nc.dram_tensor("emb", [V, H], F32, kind="ExternalInput").ap()
    wsT_t = nc.dram_tensor("wsT", [L, KT, P, 3 * H], F16, kind="ExternalInput").ap()
    bsg_t = nc.dram_tensor("bsg", [P, L * 12], F32, kind="ExternalInput").ap()
    bshalf_t = nc.dram_tensor("bshalf", [P, L * 4], F32, kind="ExternalInput").ap()
    lnb1_t = nc.dram_tensor("lnb1", [1, (L + 1) * H], F16, kind="ExternalInput").ap()
    lnw1_t = nc.dram_tensor("lnw1", [1, (L + 1) * H], F16, kind="ExternalInput").ap()
    fcwt_t = nc.dram_tensor("fcwt", [25, P, 10, KT, P], F16, kind="ExternalInput").ap()
    fcb_t = nc.dram_tensor("fcb", [P, V // P], F32, kind="ExternalInput").ap()
    rst_t = nc.dram_tensor("rst", [P, 2], F32, kind="ExternalInput").ap()
    out_t = nc.dram_tensor("out", [V, CHUNK], F16, kind="ExternalOutput").ap()

    with tile.TileContext(nc) as tc:
        with tc.tile_pool(name="singles", bufs=1) as singles, \
             tc.tile_pool(name="persist", bufs=1) as persist, \
             tc.tile_pool(name="fcw", bufs=fcwb) as fcwp:

            # ---- constants / small inputs ----
            idx = singles.tile([P, NG], I32)
            nc.sync.dma_start(out=idx[:], in_=idx_t[:])
            bsg = singles.tile([P, L * 12], F32)
            nc.sync.dma_start(out=bsg[:], in_=bsg_t[:])
            bshalf = singles.tile([P, L * 4], F32)
            nc.sync.dma_start(out=bshalf[:], in_=bshalf_t[:])
            lnb1 = singles.tile([1, (L + 1) * H], F16)
            nc.sync.dma_start(out=lnb1[:], in_=lnb1_t[:])
            lnw1 = singles.tile([1, (L + 1) * H], F16)
            nc.sync.dma_start(out=lnw1[:], in_=lnw1_t[:])
            rst = singles.tile([P, 2], F32)
            nc.sync.dma_start(out=rst[:], in_=rst_t[:])
            ident = singles.tile([P, P], F32)
            make_identity(nc, ident[:])
            ones16 = singles.tile([P, 1], F16)   # stats-reduce lhsT
            nc.vector.memset(ones16[:], 1.0)
            onesrow = singles.tile([1, W], F16)   # B0 bias-broadcast rhs
            nc.vector.memset(onesrow[:], 1.0)
            eps_t = singles.tile([1, 1], F32)
            nc.vector.memset(eps_t[:], EPS)
            fcb2 = singles.tile([P, V // P], F32)
            nc.sync.dma_start(out=fcb2[:], in_=fcb_t[:])

            # x tiles (channel-major activations), rotate per layer
            xf_bf = [persist.tile([P, CHUNK], F16, tag=f"xfbf{k}", name=f"xfbf{k}")
                     for k in range(KT)]

            with tc.tile_pool(name="xpool", bufs=2) as xpool, \
                 tc.tile_pool(name="wst", bufs=wstb) as wstp, \
                 tc.tile_pool(name="work", bufs=workb) as work, \
                 tc.tile_pool(name="scan", bufs=1) as scanp, \
                 tc.tile_pool(name="stat", bufs=statb) as statp, \
                 tc.tile_pool(name="xnp", bufs=1) as xnp, \
                 tc.tile_pool(name="psg", bufs=psgb, space="PSUM") as psg, \
                 tc.tile_pool(name="pss", bufs=1, space="PSUM") as pss, \
                 tc.tile_pool(name="psb", bufs=psbb, space="PSUM") as psb:

                # ---- phase A: embedding gather + transpose to channel-major
                x = [xpool.tile([P, W], F16, tag=f"x{k}", name=f"xt{k}") for k in range(KT)]
                for g in range(NG):
                    xg = work.tile([P, H], F32, tag="xg")
                    nc.gpsimd.indirect_dma_start(
                        out=xg[:], out_offset=None, in_=emb_t[:],
                        in_offset=IndirectOffsetOnAxis(ap=idx[:, g:g + 1], axis=0),
                    )
                    for k in range(KT):
                        ptr = psg.tile([P, 512], F32, tag="pg", name="ptr")
                        nc.tensor.transpose(
                            out=ptr[:, :P], in_=xg[:, k * P:(k + 1) * P],
                            identity=ident[:])
                        nc.scalar.copy(out=x[k][:, g * P:(g + 1) * P], in_=ptr[:, :P])

                # ---- phase B: L recurrent layers ----
                for l in range(L if not skip_phase1 else 0):
                    wst = wstp.tile([P, KT * 3 * H], F16, tag="wst")
                    for kk in range(KT):
                        nc.sync.dma_start(
                            out=wst[:, kk * 3 * H:(kk + 1) * 3 * H],
                            in_=wsT_t[l, kk])

                    # --- LayerNorm (stats over channels = partition dim) ---
                    mneg = statp.tile([1, W], F32, tag="mneg")
                    ex2 = statp.tile([1, W], F32, tag="ex2")
                    rstd = statp.tile([1, W], F32, tag="rstd")
                    rstd16 = statp.tile([1, W], F16, tag="rstd16")
                    mr16 = statp.tile([1, W], F16, tag="mr16")
                    scr = statp.tile([1, W], F32, tag="scr")
                    scr2 = statp.tile([1, W], F32, tag="scr2")
                    nrt = statp.tile([1, W], F32, tag="nrt")
                    xn = [xnp.tile([P, W], F16, tag=f"xn{k}", name=f"xn{k}") for k in range(KT)]

                    for (o, n) in NCH:
                        ps_a = pss.tile([1, 512], F32, tag="ps_a")
                        ps_b = pss.tile([1, 512], F32, tag="ps_b")
                        for k in range(KT):
                            sq = work.tile([P, 512], F16, tag="sq")
                            nc.scalar.square(out=sq[:, :n], in_=x[k][:, o:o + n])
                            nc.tensor.matmul(
                                out=ps_a[:, :n], lhsT=(ones16[:]),
                                rhs=(x[k][:, o:o + n]),
                                start=(k == 0), stop=(k == KT - 1))
                            nc.tensor.matmul(
                                out=ps_b[:, :n], lhsT=(ones16[:]),
                                rhs=(sq[:, :n]),
                                start=(k == 0), stop=(k == KT - 1))
                        nc.scalar.mul(out=mneg[0:1, o:o + n], in_=ps_a[0:1, :n],
                                      mul=-1.0 / H)
                        nc.scalar.mul(out=ex2[0:1, o:o + n], in_=ps_b[0:1, :n],
                                      mul=1.0 / H)
                        # var = E[x^2] - mean^2 ; se = sqrt(var+eps); rstd = 1/se
                        nc.vector.tensor_mul(scr[0:1, o:o + n], mneg[0:1, o:o + n],
                                             mneg[0:1, o:o + n])
                        nc.vector.tensor_sub(scr[0:1, o:o + n], ex2[0:1, o:o + n],
                                             scr[0:1, o:o + n])
                    # merged rstd = 1/sqrt(var+eps) chain over the full window
                    sl = slice(0, W)
                    nc.vector.tensor_scalar_add(scr[0:1, sl], scr[0:1, sl], EPS)
                    nc.vector.tensor_scalar_mul(scr2[0:1, sl], scr[0:1, sl],
                                                -0.5)
                    nc.vector.tensor_scalar(
                        out=rstd[0:1, sl].bitcast(I32),
                        in0=scr[0:1, sl].bitcast(I32),
                        scalar1=1, scalar2=None,
                        op0=OP.arith_shift_right)
                    nc.vector.tensor_scalar(
                        out=rstd[0:1, sl].bitcast(I32),
                        in0=rstd[0:1, sl].bitcast(I32),
                        scalar1=0x5F3759DF, scalar2=None,
                        op0=OP.subtract)
                    nc.vector.tensor_scalar_mul(rstd[0:1, sl].bitcast(I32),
                                                rstd[0:1, sl].bitcast(I32), -1)
                    for _ in range(2):
                        nc.vector.tensor_mul(nrt[0:1, sl], rstd[0:1, sl],
                                             rstd[0:1, sl])
                        nc.vector.tensor_mul(nrt[0:1, sl], nrt[0:1, sl],
                                             scr2[0:1, sl])
                        nc.vector.scalar_tensor_tensor(
                            out=rstd[0:1, sl], in0=nrt[0:1, sl], scalar=1.5,
                            in1=rstd[0:1, sl], op0=OP.add, op1=OP.mult)
                    nc.scalar.copy(out=rstd16[0:1, sl], in_=rstd[0:1, sl])
                    nc.vector.tensor_mul(scr[0:1, sl], mneg[0:1, sl],
                                         rstd[0:1, sl])
                    nc.scalar.copy(out=mr16[0:1, sl], in_=scr[0:1, sl])
                    # xn = x * (lnw x rstd) + (lnb x 1 + lnw x (-m*rstd))
                    for (o, n) in NCH:
                        for k in range(KT):
                            lsl = slice(l * H + k * P, l * H + (k + 1) * P)
                            b1 = psb.tile([P, 512], F32, tag="b1")
                            nc.tensor.matmul(out=b1[:, :n],
                                             lhsT=(lnw1[0:1, lsl]),
                                             rhs=(rstd16[0:1, o:o + n]),
                                             start=True, stop=True)
                            b0 = psb.tile([P, 512], F32, tag="b0")
                            nc.tensor.matmul(out=b0[:, :n],
                                             lhsT=(lnb1[0:1, lsl]),
                                             rhs=(onesrow[0:1, o:o + n]),
                                             start=True, stop=False)
                            nc.tensor.matmul(out=b0[:, :n],
                                             lhsT=(lnw1[0:1, lsl]),
                                             rhs=(mr16[0:1, o:o + n]),
                                             start=False, stop=True)
                            nc.vector.tensor_mul(xn[k][:, o:o + n], b1[:, :n],
                                                 x[k][:, o:o + n])
                            nc.vector.tensor_add(xn[k][:, o:o + n], b0[:, :n],
                                                 xn[k][:, o:o + n])

                    # --- gates GEMM + nonlinearities + scan ---
                    fp = [scanp.tile([P, W], F16, tag=f"fp{k}", name=f"fp{k}") for k in range(KT)]
                    vv = [scanp.tile([P, W], F16, tag=f"vv{k}", name=f"vv{k}") for k in range(KT)]
                    hh = [scanp.tile([P, W], F16, tag=f"h{k}", name=f"h{k}") for k in range(KT)]
                    x2 = [xpool.tile([P, W], F16, tag=f"x{k}", name=f"xt{k}") for k in range(KT)]
                    for (o, n) in NCH:
                        for k in range(KT):
                            def gate_mm(gate):
                                pg = psg.tile([P, 512], F32, tag="pg")
                                for kk in range(KT):
                                    c0 = kk * 3 * H + gate * H + k * P
                                    nc.tensor.matmul(
                                        out=pg[:, :n],
                                        lhsT=(wst[:, c0:c0 + P]),
                                        rhs=(xn[kk][:, o:o + n]),
                                        start=(kk == 0), stop=(kk == KT - 1))
                                return pg

                            # f' = sig(f)/(sig(f)+sig(i)), v = sig(i)*g/(..)
                            pg_f = gate_mm(0)
                            sf = work.tile([P, 512], F32, tag="sf")
                            nc.scalar.activation(
                                out=sf[:, :n], in_=pg_f[:, :n], func=AF.Sigmoid,
                                bias=bsg[:, l * 12 + k:l * 12 + k + 1])
                            pg_i = gate_mm(1)
                            si = work.tile([P, 512], F32, tag="si")
                            nc.scalar.activation(
                                out=si[:, :n], in_=pg_i[:, :n], func=AF.Sigmoid,
                                bias=bsg[:, l * 12 + 4 + k:l * 12 + 4 + k + 1])
                            ssum = work.tile([P, 512], F32, tag="ssum")
                            nc.gpsimd.tensor_add(ssum[:, :n], sf[:, :n], si[:, :n])
                            rinv = work.tile([P, 512], F32, tag="rinv")
                            nc.vector.reciprocal_approx_fast(
                                out=rinv[:, :n], in_=ssum[:, :n])
                            nc.gpsimd.tensor_mul(fp[k][:, o:o + n], sf[:, :n],
                                                 rinv[:, :n])
                            pg_t = gate_mm(2)
                            sg = work.tile([P, 512], F32, tag="sg")
                            nc.scalar.activation(
                                out=sg[:, :n], in_=pg_t[:, :n], func=AF.Sigmoid,
                                bias=bsg[:, l * 12 + 8 + k:l * 12 + 8 + k + 1])
                            g = work.tile([P, 512], F32, tag="g")
                            nc.vector.scalar_tensor_tensor(
                                out=g[:, :n], in0=pg_t[:, :n],
                                scalar=bshalf[:, l * 4 + k:l * 4 + k + 1],
                                in1=sg[:, :n], op0=OP.add, op1=OP.max)
                            num = work.tile([P, 512], F32, tag="num")
                            nc.vector.tensor_mul(num[:, :n], si[:, :n], g[:, :n])
                            nc.vector.tensor_mul(vv[k][:, o:o + n], num[:, :n],
                                                 rinv[:, :n])
                            if o == 0:
                                # boundary reset at own-region start (exact
                                # for j==0 cores)
                                t1 = work.tile([P, 1], F32, tag="t1")
                                nc.vector.tensor_mul(
                                    t1[:], fp[k][:, HALO:HALO + 1], rst[:, 1:2])
                                nc.vector.tensor_add(
                                    vv[k][:, HALO:HALO + 1], t1[:],
                                    vv[k][:, HALO:HALO + 1])
                                nc.vector.tensor_mul(
                                    fp[k][:, HALO:HALO + 1],
                                    fp[k][:, HALO:HALO + 1], rst[:, 0:1])
                            else:
                                nc.vector.tensor_tensor_scan(
                                    out=hh[k][:], data0=fp[k][:],
                                    data1=vv[k][:],
                                    initial=0.5, op0=OP.mult, op1=OP.add)
                                nc.vector.tensor_add(x2[k][:], hh[k][:],
                                                     x[k][:])
                    x = x2

                # ---- final LayerNorm (own tokens only) + bf16 cast ----
                if skip_phase1:
                    for k in range(KT):
                        nc.vector.memset(xf_bf[k][:], 0.01)
                o, n = OWN
                if skip_phase1:
                    n = 0  # handled via memset above
                el_ln = not skip_phase1

                if el_ln:
                  mneg = statp.tile([1, 512], F32, tag="mneg")
                  ex2 = statp.tile([1, 512], F32, tag="ex2")
                  rstd = statp.tile([1, 512], F32, tag="rstd")
                  rstd16 = statp.tile([1, 512], F16, tag="rstd16")
                  mr16 = statp.tile([1, 512], F16, tag="mr16")
                  scr = statp.tile([1, 512], F32, tag="scr")
                  scr2 = statp.tile([1, 512], F32, tag="scr2")
                  nrt = statp.tile([1, 512], F32, tag="nrt")
                  ps_a = pss.tile([1, 512], F32, tag="ps_a")
                  ps_b = pss.tile([1, 512], F32, tag="ps_b")
                  for k in range(KT):
                      sq = work.tile([P, 512], F16, tag="sq")
                      nc.scalar.square(out=sq[:, :n], in_=x[k][:, o:o + n])
                      nc.tensor.matmul(out=ps_a[:, :n], lhsT=(ones16[:]),
                                       rhs=(x[k][:, o:o + n]),
                                       start=(k == 0), stop=(k == KT - 1))
                      nc.tensor.matmul(out=ps_b[:, :n], lhsT=(ones16[:]),
                                       rhs=(sq[:, :n]),
                                       start=(k == 0), stop=(k == KT - 1))
                  nc.scalar.mul(out=mneg[0:1, :n], in_=ps_a[0:1, :n], mul=-1.0 / H)
                  nc.scalar.mul(out=ex2[0:1, :n], in_=ps_b[0:1, :n], mul=1.0 / H)
                  nc.vector.tensor_mul(scr[0:1, :n], mneg[0:1, :n], mneg[0:1, :n])
                  nc.vector.tensor_sub(scr[0:1, :n], ex2[0:1, :n], scr[0:1, :n])
                  nc.vector.tensor_scalar_add(scr[0:1, :n], scr[0:1, :n], EPS)
                  nc.vector.tensor_scalar_mul(scr2[0:1, :n], scr[0:1, :n], -0.5)
                  nc.vector.tensor_scalar(
                      out=rstd[0:1, :n].bitcast(I32),
                      in0=scr[0:1, :n].bitcast(I32),
                      scalar1=1, scalar2=None,
                      op0=OP.arith_shift_right)
                  nc.vector.tensor_scalar(
                      out=rstd[0:1, :n].bitcast(I32),
                      in0=rstd[0:1, :n].bitcast(I32),
                      scalar1=0x5F3759DF, scalar2=None,
                      op0=OP.subtract)
                  nc.vector.tensor_scalar_mul(rstd[0:1, :n].bitcast(I32),
                                              rstd[0:1, :n].bitcast(I32), -1)
                  for _ in range(2):
                      nc.vector.tensor_mul(nrt[0:1, :n], rstd[0:1, :n], rstd[0:1, :n])
                      nc.vector.tensor_mul(nrt[0:1, :n], nrt[0:1, :n], scr2[0:1, :n])
                      nc.vector.scalar_tensor_tensor(
                          out=rstd[0:1, :n], in0=nrt[0:1, :n], scalar=1.5,
                          in1=rstd[0:1, :n], op0=OP.add, op1=OP.mult)
                  nc.scalar.copy(out=rstd16[0:1, :n], in_=rstd[0:1, :n])
                  nc.vector.tensor_mul(scr[0:1, :n], mneg[0:1, :n], rstd[0:1, :n])
                  nc.scalar.copy(out=mr16[0:1, :n], in_=scr[0:1, :n])
                  for k in range(KT):
                      lsl = slice(L * H + k * P, L * H + (k + 1) * P)
                      b1 = psb.tile([P, 512], F32, tag="b1")
                      nc.tensor.matmul(out=b1[:, :n], lhsT=(lnw1[0:1, lsl]),
                                       rhs=(rstd16[0:1, :n]), start=True, stop=True)
                      b0 = psb.tile([P, 512], F32, tag="b0")
                      nc.tensor.matmul(out=b0[:, :n], lhsT=(lnb1[0:1, lsl]),
                                       rhs=(onesrow[0:1, :n]), start=True, stop=False)
                      nc.tensor.matmul(out=b0[:, :n], lhsT=(lnw1[0:1, lsl]),
                                       rhs=(mr16[0:1, :n]), start=False, stop=True)
                      xf = work.tile([P, 512], F32, tag="xf")
                      nc.vector.tensor_mul(xf[:, :n], b1[:, :n], x[k][:, o:o + n])
                      nc.vector.tensor_add(xf_bf[k][:], b0[:, :n], xf[:, :n])

            # ---- phase C: logits GEMM (own 512 tokens x full vocab) ----
            if skip_phasec:
                for k in range(KT):
                    nc.sync.dma_start(out=out_t[k * P:(k + 1) * P, 0:512],
                                      in_=xf_bf[k][:])
            VG = 10   # vocab tiles per fcw load (25 groups of 10)
            with tc.tile_pool(name="osb", bufs=8) as osbp, \
                 tc.tile_pool(name="pso", bufs=8, space="PSUM") as pso:
                for vg in range(25 if not skip_phasec else 0):
                    fcw = fcwp.tile([P, VG, KT, P], F16, tag="fcw")
                    (nc.sync if ldsync else nc.gpsimd).dma_start(out=fcw[:], in_=fcwt_t[vg])
                    for j in range(VG):
                        vt = vg * VG + j
                        po = pso.tile([P, CHUNK], F32, tag="po")
                        for k in range(KT):
                            nc.tensor.matmul(
                                out=po[:], lhsT=fcw[:, j, k, :],
                                rhs=xf_bf[k][:],
                                start=(k == 0), stop=(k == KT - 1))
                        osb = osbp.tile([P, CHUNK], F16, tag="osb")
                        if vt % 2 == 0:
                            nc.scalar.activation(out=osb[:], in_=po[:],
                                                 func=AF.Identity,
                                                 bias=fcb2[:, vt:vt + 1])
                        else:
                            nc.vector.tensor_scalar(
                                out=osb[:], in0=po[:],
                                scalar1=fcb2[:, vt:vt + 1], scalar2=None,
                                op0=OP.add)
                        (nc.sync if vt % 2 == 0 else nc.scalar).dma_start(
                            out=out_t[vt * P:(vt + 1) * P, :], in_=osb[:])

    nc.compile()
    return nc


_CACHED = None


def _get_program():
    global _CACHED
    if _CACHED is None:
        _CACHED = build_program()
    return _CACHED


def prep_inputs(ids, emb, Ws, bs, ln_w, ln_b, fln_w, fc_w, fc_b):
    """Host-side layout prep -> per-core input maps."""
    ids = np.asarray(ids)
    emb = np.ascontiguousarray(np.asarray(emb, dtype=np.float32))
    Ws = np.asarray(Ws, dtype=np.float32)
    bs = np.asarray(bs, dtype=np.float32)
    ln_w = np.asarray(ln_w, dtype=np.float32)
    ln_b = np.asarray(ln_b, dtype=np.float32)
    fln_w = np.asarray(fln_w, dtype=np.float32)
    fc_w = np.asarray(fc_w, dtype=np.float32)
    fc_b = np.asarray(fc_b, dtype=np.float32)

    # Ws[l].T tiled into [KT, 128, 3H], fp16
    wsT = np.ascontiguousarray(
        np.stack([Ws[l].T.reshape(KT, P, 3 * H) for l in range(L)])).astype(
            np.float16)

    # per-partition gate biases, grouped [l][gate][k]
    bsg = np.empty((P, L * 12), np.float32)
    bshalf = np.empty((P, L * 4), np.float32)
    for l in range(L):
        for gate in range(3):
            for k in range(KT):
                bsg[:, l * 12 + gate * 4 + k] =                     bs[l, gate * H + k * P:gate * H + (k + 1) * P]
        for k in range(KT):
            bshalf[:, l * 4 + k] = bs[l, 2 * H + k * P:2 * H + (k + 1) * P] + 0.5

    lnb1 = np.zeros((1, (L + 1) * H), np.float32)
    lnw1 = np.zeros((1, (L + 1) * H), np.float32)
    for l in range(L):
        lnb1[0, l * H:(l + 1) * H] = ln_b[l]
        lnw1[0, l * H:(l + 1) * H] = ln_w[l]
    lnw1[0, L * H:] = fln_w

    # fc_w.T tiled [25, 128, 10, KT, 128] fp16, vocab-major logits layout
    fcwt = np.ascontiguousarray(
        fc_w.T.reshape(KT, P, 25, 10, P).transpose(2, 1, 3, 0, 4)).astype(
            np.float16)
    fcb2 = np.ascontiguousarray(fc_b.reshape(V // P, P).T)

    shared = {"emb": emb, "wsT": wsT, "bsg": bsg,
              "bshalf": bshalf, "lnb1": lnb1.astype(np.float16),
              "lnw1": lnw1.astype(np.float16),
              "fcwt": fcwt, "fcb": fcb2}

    in_maps = []
    for c in range(N_CORES):
        b, j = divmod(c, 4)
        own0 = j * CHUNK
        win = np.zeros(W, np.int32)
        if j == 0:
            win[HALO:] = ids[b, :CHUNK]
        else:
            win[:] = ids[b, own0 - HALO:own0 + CHUNK]
        idxt = np.ascontiguousarray(win.reshape(NG, P).T)
        rstc = np.empty((P, 2), np.float32)
        rstc[:, 0] = 0.0 if j == 0 else 1.0   # multiplies f at window pos HALO
        rstc[:, 1] = 0.5 if j == 0 else 0.0   # adds f*this to v at pos HALO
        in_maps.append({**shared, "idx": idxt, "rst": rstc})
    return in_maps


def kernel(ids, emb, Ws, bs, ln_w, ln_b, fln_w, fc_w, fc_b):
    nc = _get_program()
    in_maps = prep_inputs(ids, emb, Ws, bs, ln_w, ln_b, fln_w, fc_w, fc_b)
    res = run_bass_kernel_spmd(nc, in_maps, list(range(N_CORES)))
    out = np.empty((B, S, V), np.float32)
    for c in range(N_CORES):
        b, j = divmod(c, 4)
        out[b, j * CHUNK:(j + 1) * CHUNK, :] = \
            res.results[c]["out"].T.astype(np.float32)
    return out



# revision 10
# speedup vs baseline: 1.2096x; 1.1193x over previous
"""Trainium2 Bass kernel for a 3-layer minLSTM-style NLP model.

Model (per reference):
  x = emb[ids]                                   (B,S,E) = (2,2048,512)
  3 x { xn = LN(x); gates = xn @ Ws.T + bs;
        f' = sig(f)/(sig(f)+sig(i)); i' = 1-f';
        v = i' * g(tilde), g(x) = max(x+0.5, sigmoid(x));
        h_t = f'_t h_{t-1} + v_t  (h_0 = 0.5);  x = h + x }
  xf = LN(x) * fln_w;  logits = xf @ fc_w.T + fc_b    (B,S,32000)

Sharding (8 cores, zero collectives):
  core c -> (batch b=c//4, seq chunk j=c%4 of 512 tokens). Each core runs a
  640-token window (128-token halo before its own 512) through the recurrent
  stack; the forget-product decays the unknown initial state to ~0 over the
  halo, and a per-core reset constant makes j==0 exact at the batch start.
  Each core computes logits for its own 512 tokens against the full vocab.

Key layout tricks:
  - LN affine (ln_w/ln_b) folded into the gate weights/biases on host, and
    fln_w folded into fc_w, so the device only applies (x-m)*rstd.
  - Per-token LN stats computed TRANSPOSED (tokens on partitions) via 1-row
    matmuls; rsqrt via magic-constant Newton on tiny [128,5] tiles; result
    transposed back and broadcast across partitions by the Pool engine.
  - Activations f16 everywhere; output logits written f16 and upcast on host.
"""

import sys

if "/opt/trn_rl_repo" not in sys.path:
    sys.path.insert(0, "/opt/trn_rl_repo")

import numpy as np

import concourse.bass as bass
import concourse.bacc as bacc
import concourse.tile as tile
from concourse import mybir
from concourse.bass import IndirectOffsetOnAxis
from concourse.bass_utils import run_bass_kernel_spmd
from concourse.masks import make_identity

F32 = mybir.dt.float32
F16 = mybir.dt.float16
I32 = mybir.dt.int32
AF = mybir.ActivationFunctionType
OP = mybir.AluOpType

# problem constants
B, S, V, H, L = 2, 2048, 32000, 512, 3
P = 128
KT = H // P            # 4 k-tiles over the H contraction dim
CHUNK = 512            # own tokens per core
HALO = 128             # speculative scan warmup tokens
W = HALO + CHUNK       # 640 window tokens per core
NG = W // P            # 5 embedding gather groups
NCH = [(0, 512), (512, 128)]   # window free-dim chunks (PSUM-bounded)
VC = 500               # vocab chunk for logits
N_CORES = 8
EPS = 1e-5
MAGIC2 = 0x1EF759DF    # rsqrt seed magic, pre-adjusted for hneg=-(var+eps)/2


def build_program(fcwb=9, psgb=4, workb=2, wstb=2):
    nc = bacc.Bacc("TRN2", target_bir_lowering=False, debug=False,
                   enable_asserts=True, num_devices=N_CORES)

    idx_t = nc.dram_tensor("idx", [P, NG], I32, kind="ExternalInput").ap()
    emb_t = nc.dram_tensor("emb", [V, H], F16, kind="ExternalInput").ap()
    wsT_t = nc.dram_tensor("wsT", [L, KT, P, 3 * H], F16, kind="ExternalInput").ap()
    bsg_t = nc.dram_tensor("bsg", [P, L * 12], F32, kind="ExternalInput").ap()
    bshalf_t = nc.dram_tensor("bshalf", [P, L * 4], F32, kind="ExternalInput").ap()
    fcwt_t = nc.dram_tensor("fcwt", [25, P, 10, KT, P], F16, kind="ExternalInput").ap()
    fcb_t = nc.dram_tensor("fcb", [P, V // P], F32, kind="ExternalInput").ap()
    rst_t = nc.dram_tensor("rst", [P, 2], F32, kind="ExternalInput").ap()
    out_t = nc.dram_tensor("out", [V, CHUNK], F16, kind="ExternalOutput").ap()

    with tile.TileContext(nc) as tc:
        with tc.tile_pool(name="singles", bufs=1) as singles, \
             tc.tile_pool(name="persist", bufs=1) as persist, \
             tc.tile_pool(name="fcw", bufs=fcwb) as fcwp:

            # ---- constants / small inputs ----
            idx = singles.tile([P, NG], I32)
            nc.sync.dma_start(out=idx[:], in_=idx_t[:])
            bsg = singles.tile([P, L * 12], F32)
            nc.sync.dma_start(out=bsg[:], in_=bsg_t[:])
            bshalf = singles.tile([P, L * 4], F32)
            nc.sync.dma_start(out=bshalf[:], in_=bshalf_t[:])
            rst = singles.tile([P, 2], F32)
            nc.sync.dma_start(out=rst[:], in_=rst_t[:])
            fcb2 = singles.tile([P, V // P], F32)
            nc.sync.dma_start(out=fcb2[:], in_=fcb_t[:])
            ident16 = singles.tile([P, P], F16)
            make_identity(nc, ident16[:])
            ones16 = singles.tile([P, 1], F16)   # stats-reduce rhs
            nc.vector.memset(ones16[:], 1.0)

            # final activations (channel-major), consumed by phase C
            xf_bf = [persist.tile([P, CHUNK], F16, tag=f"xfbf{k}", name=f"xfbf{k}")
                     for k in range(KT)]

            with tc.tile_pool(name="xpool", bufs=2) as xpool, \
                 tc.tile_pool(name="wst", bufs=wstb) as wstp, \
                 tc.tile_pool(name="work", bufs=workb) as work, \
                 tc.tile_pool(name="scan", bufs=1) as scanp, \
                 tc.tile_pool(name="sq", bufs=1) as sqp, \
                 tc.tile_pool(name="xnp", bufs=1) as xnp, \
                 tc.tile_pool(name="bc", bufs=1) as bcp, \
                 tc.tile_pool(name="stat", bufs=1) as statp, \
                 tc.tile_pool(name="psg", bufs=psgb, space="PSUM") as psg, \
                 tc.tile_pool(name="pss", bufs=1, space="PSUM") as pss, \
                 tc.tile_pool(name="pst", bufs=2, space="PSUM") as pst:

                # ---- phase A: embedding gather + transpose to channel-major
                x = [xpool.tile([P, W], F16, tag=f"x{k}", name=f"xt{k}") for k in range(KT)]
                for g in range(NG):
                    xg = work.tile([P, H], F16, tag="xg")
                    nc.gpsimd.indirect_dma_start(
                        out=xg[:], out_offset=None, in_=emb_t[:],
                        in_offset=IndirectOffsetOnAxis(ap=idx[:, g:g + 1], axis=0),
                    )
                    for k in range(KT):
                        ptr = pst.tile([P, P], F16, tag="pstt", name="ptr")
                        nc.tensor.transpose(
                            out=ptr[:], in_=xg[:, k * P:(k + 1) * P],
                            identity=ident16[:])
                        eng = nc.vector if (g * KT + k) % 2 == 0 else nc.scalar
                        if eng is nc.vector:
                            nc.vector.tensor_copy(
                                out=x[k][:, g * P:(g + 1) * P], in_=ptr[:])
                        else:
                            nc.scalar.copy(
                                out=x[k][:, g * P:(g + 1) * P], in_=ptr[:])

                # ---- helper: transposed LN stats + rsqrt newton ----
                def ln_stats(xs, sqs, g0, ngr, tag):
                    """Per-token -mean*rstd and rstd for token groups
                    [g0, g0+ngr) of the window, returned as a [2*ngr, P] f16
                    SBUF tile: rows [0,ngr) = rstd, rows [ngr, 2*ngr) = mr."""
                    psT = pss.tile([P, 2 * ngr], F32, tag=f"psT{tag}", name="psT")
                    for g in range(ngr):
                        for k in range(KT):
                            nc.tensor.matmul(
                                out=psT[:, g:g + 1],
                                lhsT=xs[k][:, (g0 + g) * P:(g0 + g + 1) * P],
                                rhs=ones16[:],
                                start=(k == 0), stop=(k == KT - 1))
                    for g in range(ngr):
                        for k in range(KT):
                            nc.tensor.matmul(
                                out=psT[:, ngr + g:ngr + g + 1],
                                lhsT=sqs[k][:, (g0 + g) * P:(g0 + g + 1) * P],
                                rhs=ones16[:],
                                start=(k == 0), stop=(k == KT - 1))
                    m2 = statp.tile([P, ngr], F32, tag="m2")
                    hneg = statp.tile([P, ngr], F32, tag="hneg")
                    y = statp.tile([P, ngr], F32, tag="y")
                    t = statp.tile([P, ngr], F32, tag="t")
                    rsmr = statp.tile([P, 2 * ngr], F16, tag="rsmr")
                    # m2 = (sum_x/(H*sqrt2))^2 = m^2/2
                    nc.scalar.activation(out=m2[:], in_=psT[:, 0:ngr],
                                         func=AF.Square,
                                         scale=1.0 / (H * np.sqrt(2.0)))
                    # hneg = m^2/2 - (sum_x2/(2H) + eps/2) = -(var+eps)/2
                    nc.vector.tensor_scalar(
                        out=hneg[:], in0=psT[:, ngr:2 * ngr],
                        scalar1=0.5 / H, scalar2=EPS / 2,
                        op0=OP.mult, op1=OP.add)
                    nc.vector.tensor_sub(hneg[:], m2[:], hneg[:])
                    # rsqrt seed: y = -( (bits(hneg)>>1) - MAGIC2 )
                    nc.vector.tensor_scalar(
                        out=y[:].bitcast(I32), in0=hneg[:].bitcast(I32),
                        scalar1=1, scalar2=None,
                        op0=OP.arith_shift_right)
                    nc.vector.tensor_scalar(
                        out=y[:].bitcast(I32), in0=y[:].bitcast(I32),
                        scalar1=MAGIC2, scalar2=None, op0=OP.subtract)
                    nc.vector.tensor_scalar_mul(y[:].bitcast(I32),
                                                y[:].bitcast(I32), -1)
                    for _ in range(2):
                        nc.vector.tensor_mul(t[:], y[:], y[:])
                        nc.vector.tensor_mul(t[:], t[:], hneg[:])
                        nc.vector.scalar_tensor_tensor(
                            out=y[:], in0=t[:], scalar=1.5, in1=y[:],
                            op0=OP.add, op1=OP.mult)
                    nc.vector.tensor_copy(out=rsmr[:, 0:ngr], in_=y[:])
                    # mr = -(sum_x/H)*rstd
                    nc.vector.scalar_tensor_tensor(
                        out=rsmr[:, ngr:2 * ngr], in0=psT[:, 0:ngr],
                        scalar=-1.0 / H, in1=y[:], op0=OP.mult, op1=OP.mult)
                    # transpose each column separately so every row lands
                    # on partition 0 (partition_broadcast requirement)
                    rows = statp.tile([1, 2 * NG * P], F16, tag="rows",
                                      name="rows")
                    for q in range(2 * ngr):
                        ptrq = pst.tile([1, P], F16, tag="pstt", name="ptrq")
                        nc.tensor.transpose(out=ptrq[:], in_=rsmr[:, q:q + 1],
                                            identity=ident16[:])
                        if q % 2 == 0:
                            nc.vector.tensor_copy(
                                out=rows[0:1, q * P:(q + 1) * P], in_=ptrq[:])
                        else:
                            nc.scalar.copy(
                                out=rows[0:1, q * P:(q + 1) * P], in_=ptrq[:])
                    return rows

                # ---- phase B: L recurrent layers ----
                for l in range(L):
                    wst = wstp.tile([P, KT * 3 * H], F16, tag="wst")
                    for kk in range(KT):
                        nc.sync.dma_start(
                            out=wst[:, kk * 3 * H:(kk + 1) * 3 * H],
                            in_=wsT_t[l, kk])

                    sq = [sqp.tile([P, W], F16, tag=f"sq{k}", name=f"sq{k}") for k in range(KT)]
                    for k in range(KT):
                        nc.gpsimd.tensor_mul(sq[k][:], x[k][:], x[k][:])

                    rows = ln_stats(x, sq, 0, NG, "b")

                    # broadcast rstd/mr across partitions (Pool engine)
                    rb = bcp.tile([P, W], F16, tag="rb")
                    mb = bcp.tile([P, W], F16, tag="mb")
                    for g in range(NG):
                        nc.gpsimd.partition_broadcast(
                            rb[:, g * P:(g + 1) * P],
                            rows[0:1, g * P:(g + 1) * P])
                        nc.gpsimd.partition_broadcast(
                            mb[:, g * P:(g + 1) * P],
                            rows[0:1, (NG + g) * P:(NG + g + 1) * P])

                    # xn = (x - m) * rstd  (affine folded into weights)
                    xn = [xnp.tile([P, W], F16, tag=f"xn{k}", name=f"xn{k}") for k in range(KT)]
                    for k in range(KT):
                        nc.vector.tensor_mul(xn[k][:], x[k][:], rb[:])
                        nc.vector.tensor_add(xn[k][:], xn[k][:], mb[:])

                    # --- gates GEMM + nonlinearities + scan ---
                    fp = [scanp.tile([P, W], F16, tag=f"fp{k}", name=f"fp{k}") for k in range(KT)]
                    vv = [scanp.tile([P, W], F16, tag=f"vv{k}", name=f"vv{k}") for k in range(KT)]
                    hh = [scanp.tile([P, W], F16, tag=f"h{k}", name=f"h{k}") for k in range(KT)]
                    x2 = [xpool.tile([P, W], F16, tag=f"x{k}", name=f"xt{k}") for k in range(KT)]
                    for (o, n) in NCH:
                        for k in range(KT):
                            def gate_mm(gate):
                                pg = psg.tile([P, 512], F32, tag="pg")
                                for kk in range(KT):
                                    c0 = kk * 3 * H + gate * H + k * P
                                    nc.tensor.matmul(
                                        out=pg[:, :n],
                                        lhsT=(wst[:, c0:c0 + P]),
                                        rhs=(xn[kk][:, o:o + n]),
                                        start=(kk == 0), stop=(kk == KT - 1))
                                return pg

                            pg_f = gate_mm(0)
                            sf = work.tile([P, 512], F16, tag="sf")
                            nc.scalar.activation(
                                out=sf[:, :n], in_=pg_f[:, :n], func=AF.Sigmoid,
                                bias=bsg[:, l * 12 + k:l * 12 + k + 1])
                            pg_i = gate_mm(1)
                            si = work.tile([P, 512], F16, tag="si")
                            nc.scalar.activation(
                                out=si[:, :n], in_=pg_i[:, :n], func=AF.Sigmoid,
                                bias=bsg[:, l * 12 + 4 + k:l * 12 + 4 + k + 1])
                            ssum = work.tile([P, 512], F32, tag="ssum")
                            nc.vector.tensor_add(ssum[:, :n], sf[:, :n], si[:, :n])
                            rinv = work.tile([P, 512], F32, tag="rinv")
                            nc.vector.reciprocal_approx_fast(
                                out=rinv[:, :n], in_=ssum[:, :n])
                            nc.vector.tensor_mul(fp[k][:, o:o + n], sf[:, :n],
                                                 rinv[:, :n])
                            pg_t = gate_mm(2)
                            sg = work.tile([P, 512], F16, tag="sg")
                            nc.scalar.activation(
                                out=sg[:, :n], in_=pg_t[:, :n], func=AF.Sigmoid,
                                bias=bsg[:, l * 12 + 8 + k:l * 12 + 8 + k + 1])
                            lin = work.tile([P, 512], F16, tag="lin")
                            nc.scalar.activation(
                                out=lin[:, :n], in_=pg_t[:, :n], func=AF.Identity,
                                bias=bshalf[:, l * 4 + k:l * 4 + k + 1])
                            g16 = work.tile([P, 512], F16, tag="g16")
                            nc.vector.tensor_max(g16[:, :n], lin[:, :n], sg[:, :n])
                            ip16 = work.tile([P, 512], F16, tag="ip16")
                            nc.vector.tensor_scalar(
                                out=ip16[:, :n], in0=fp[k][:, o:o + n],
                                scalar1=-1.0, scalar2=1.0,
                                op0=OP.mult, op1=OP.add)
                            nc.vector.tensor_mul(vv[k][:, o:o + n], ip16[:, :n],
                                                 g16[:, :n])
                            if o == 0:
                                # boundary reset at own-region start (exact
                                # for j==0 cores)
                                t1 = work.tile([P, 1], F32, tag="t1")
                                nc.vector.tensor_mul(
                                    t1[:], fp[k][:, HALO:HALO + 1], rst[:, 1:2])
                                nc.vector.tensor_add(
                                    vv[k][:, HALO:HALO + 1], t1[:],
                                    vv[k][:, HALO:HALO + 1])
                                nc.vector.tensor_mul(
                                    fp[k][:, HALO:HALO + 1],
                                    fp[k][:, HALO:HALO + 1], rst[:, 0:1])
                            else:
                                nc.vector.tensor_tensor_scan(
                                    out=hh[k][:], data0=fp[k][:],
                                    data1=vv[k][:],
                                    initial=0.5, op0=OP.mult, op1=OP.add)
                                nc.vector.tensor_add(x2[k][:], hh[k][:],
                                                     x[k][:])
                    x = x2

                # ---- final LayerNorm (own tokens = groups 1..4) ----
                sqf = [sqp.tile([P, W], F16, tag=f"sq{k}", name=f"sqf{k}") for k in range(KT)]
                for k in range(KT):
                    nc.gpsimd.tensor_mul(sqf[k][:, HALO:], x[k][:, HALO:],
                                         x[k][:, HALO:])
                rows2 = ln_stats(x, sqf, 1, NG - 1, "f")
                rb2 = bcp.tile([P, CHUNK], F16, tag="rb2")
                mb2 = bcp.tile([P, CHUNK], F16, tag="mb2")
                for g in range(NG - 1):
                    nc.gpsimd.partition_broadcast(
                        rb2[:, g * P:(g + 1) * P],
                        rows2[0:1, g * P:(g + 1) * P])
                    nc.gpsimd.partition_broadcast(
                        mb2[:, g * P:(g + 1) * P],
                        rows2[0:1, (NG - 1 + g) * P:(NG + g) * P])
                for k in range(KT):
                    nc.vector.tensor_mul(xf_bf[k][:], x[k][:, HALO:], rb2[:])
                    nc.vector.tensor_add(xf_bf[k][:], xf_bf[k][:], mb2[:])

            # ---- phase C: logits GEMM (own 512 tokens x full vocab) ----
            VG = 10   # vocab tiles per fcw load (25 groups of 10)
            with tc.tile_pool(name="osb", bufs=8) as osbp, \
                 tc.tile_pool(name="pso", bufs=8, space="PSUM") as pso:
                for vg in range(25):
                    fcw = fcwp.tile([P, VG, KT, P], F16, tag="fcw")
                    nc.gpsimd.dma_start(out=fcw[:], in_=fcwt_t[vg])
                    for j in range(VG):
                        vt = vg * VG + j
                        po = pso.tile([P, CHUNK], F32, tag="po")
                        for k in range(KT):
                            nc.tensor.matmul(
                                out=po[:], lhsT=fcw[:, j, k, :],
                                rhs=xf_bf[k][:],
                                start=(k == 0), stop=(k == KT - 1))
                        osb = osbp.tile([P, CHUNK], F16, tag="osb")
                        if vt % 2 == 0:
                            nc.scalar.activation(out=osb[:], in_=po[:],
                                                 func=AF.Identity,
                                                 bias=fcb2[:, vt:vt + 1])
                        else:
                            nc.vector.tensor_scalar(
                                out=osb[:], in0=po[:],
                                scalar1=fcb2[:, vt:vt + 1], scalar2=None,
                                op0=OP.add)
                        (nc.sync if vt % 2 == 0 else nc.scalar).dma_start(
                            out=out_t[vt * P:(vt + 1) * P, :], in_=osb[:])

    nc.compile()
    return nc


_CACHED = None


def _get_program():
    global _CACHED
    if _CACHED is None:
        _CACHED = build_program()
    return _CACHED


def prep_inputs(ids, emb, Ws, bs, ln_w, ln_b, fln_w, fc_w, fc_b):
    """Host-side layout prep -> per-core input maps."""
    ids = np.asarray(ids)
    emb = np.asarray(emb, dtype=np.float32)
    Ws = np.asarray(Ws, dtype=np.float32)
    bs = np.asarray(bs, dtype=np.float32)
    ln_w = np.asarray(ln_w, dtype=np.float32)
    ln_b = np.asarray(ln_b, dtype=np.float32)
    fln_w = np.asarray(fln_w, dtype=np.float32)
    fc_w = np.asarray(fc_w, dtype=np.float32)
    fc_b = np.asarray(fc_b, dtype=np.float32)

    emb16 = np.ascontiguousarray(emb).astype(np.float16)

    # fold ln_w into the gate weights, ln_b into the gate biases
    # Ws'[l] = Ws[l] * ln_w[l][None,:]; bias'[l] = bs[l] + Ws[l] @ ln_b[l]
    wsT = np.ascontiguousarray(
        np.stack([(Ws[l] * ln_w[l][None, :]).T.reshape(KT, P, 3 * H)
                  for l in range(L)])).astype(np.float16)
    bias = np.stack([bs[l] + Ws[l] @ ln_b[l] for l in range(L)])  # [L, 3H]

    # per-partition gate biases, grouped [l][gate][k]
    bsg = np.empty((P, L * 12), np.float32)
    bshalf = np.empty((P, L * 4), np.float32)
    for l in range(L):
        for gate in range(3):
            for k in range(KT):
                bsg[:, l * 12 + gate * 4 + k] = \
                    bias[l, gate * H + k * P:gate * H + (k + 1) * P]
        for k in range(KT):
            bshalf[:, l * 4 + k] = bias[l, 2 * H + k * P:2 * H + (k + 1) * P] + 0.5

    # fold fln_w into fc_w; fc_w'.T tiled [25, 128, 10, KT, 128] f16
    fcw = fc_w * fln_w[None, :]
    fcwt = np.ascontiguousarray(
        fcw.T.reshape(KT, P, 25, 10, P).transpose(2, 1, 3, 0, 4)).astype(
            np.float16)
    fcb2 = np.ascontiguousarray(fc_b.reshape(V // P, P).T)

    shared = {"emb": emb16, "wsT": wsT, "bsg": bsg, "bshalf": bshalf,
              "fcwt": fcwt, "fcb": fcb2}

    in_maps = []
    for c in range(N_CORES):
        b, j = divmod(c, 4)
        own0 = j * CHUNK
        win = np.zeros(W, np.int32)
        if j == 0:
            win[HALO:] = ids[b, :CHUNK]
        else:
            win[:] = ids[b, own0 - HALO:own0 + CHUNK]
        idxt = np.ascontiguousarray(win.reshape(NG, P).T)
        rstc = np.empty((P, 2), np.float32)
        rstc[:, 0] = 0.0 if j == 0 else 1.0   # multiplies f at window pos HALO
        rstc[:, 1] = 0.5 if j == 0 else 0.0   # adds f*this to v at pos HALO
        in_maps.append({**shared, "idx": idxt, "rst": rstc})
    return in_maps


def kernel(ids, emb, Ws, bs, ln_w, ln_b, fln_w, fc_w, fc_b):
    nc = _get_program()
    in_maps = prep_inputs(ids, emb, Ws, bs, ln_w, ln_b, fln_w, fc_w, fc_b)
    res = run_bass_kernel_spmd(nc, in_maps, list(range(N_CORES)))
    out = np.empty((B, S, V), np.float32)
    for c in range(N_CORES):
        b, j = divmod(c, 4)
        out[b, j * CHUNK:(j + 1) * CHUNK, :] = \
            res.results[c]["out"].T.astype(np.float32)
    return out


# revision 11
# speedup vs baseline: 1.2341x; 1.0202x over previous
"""Trainium2 Bass kernel for a 3-layer minLSTM-style NLP model.

Model (per reference):
  x = emb[ids]                                   (B,S,E) = (2,2048,512)
  3 x { xn = LN(x); gates = xn @ Ws.T + bs;
        f' = sig(f)/(sig(f)+sig(i)); i' = 1-f';
        v = i' * g(tilde), g(x) = max(x+0.5, sigmoid(x));
        h_t = f'_t h_{t-1} + v_t  (h_0 = 0.5);  x = h + x }
  xf = LN(x) * fln_w;  logits = xf @ fc_w.T + fc_b    (B,S,32000)

Sharding (8 cores, zero collectives):
  core c -> (batch b=c//4, seq chunk j=c%4 of 512 tokens). Each core runs a
  640-token window (128-token halo before its own 512) through the recurrent
  stack; the forget-product decays the unknown initial state to ~0 over the
  halo, and a per-core reset constant makes j==0 exact at the batch start.
  Each core computes logits for its own 512 tokens against the full vocab.

Key layout tricks:
  - LN affine (ln_w/ln_b) folded into the gate weights/biases on host, and
    fln_w folded into fc_w, so the device only applies (x-m)*rstd.
  - Per-token LN stats computed TRANSPOSED (tokens on partitions) via 1-row
    matmuls; rsqrt via magic-constant Newton on tiny [128,5] tiles; result
    transposed back and broadcast across partitions by the Pool engine.
  - Activations f16 everywhere; output logits written f16 and upcast on host.
"""

import sys

if "/opt/trn_rl_repo" not in sys.path:
    sys.path.insert(0, "/opt/trn_rl_repo")

import numpy as np

import concourse.bass as bass
import concourse.bacc as bacc
import concourse.tile as tile
from concourse import mybir
from concourse.bass import IndirectOffsetOnAxis
from concourse.bass_utils import run_bass_kernel_spmd
from concourse.masks import make_identity
from concourse import dve_ops as _dve_ops
from concourse.dve_spec import AluOp as _DAlu, Bin as _DBin, Spec as _DSpec, \
    Src0 as _DS0, Src1 as _DS1, C0 as _DC0, C1 as _DC1, lower as _dlower, \
    _has_src1 as _dhas_src1
from concourse.dve_uop import DveOpSpec as _DveOpSpec


def _make_frac_op():
    """Custom DVE op: out = in0 / (in0 + in1), one bit-trick seed + one
    Newton pass (~1.7e-3 rel err). Fuses the minLSTM gate normalization
    f' = sig(f)/(sig(f)+sig(i)) into a single DVE instruction."""
    name = "FRAC_SIG_FAST_ANT"
    for op in _dve_ops.OPS:
        if op.name == name:
            return op
    _z = _DS0 + _DS1
    _nz = _DBin(_DAlu.BITWISE_NOT, _z, _z)
    _y0 = _nz * _DC0
    _y1 = _y0 * (_DC1 - _z * _y0)

    def _ref(in0, in1, c0, c1, c2):
        z = in0.astype(np.float32) + in1.astype(np.float32)
        nz = (~z.view(np.int32)).view(np.float32)
        y0 = nz * np.float32(c0)
        y1 = (y0 * (np.float32(c1) - z * y0)).astype(np.float32)
        return in0.astype(np.float32) * y1

    spec = _DSpec(body=_DS0 * _y1, reference=_ref)
    row = max(_dve_ops._SUB_OPCODE_FOR_NAME.values()) + 1
    _dve_ops._SUB_OPCODE_FOR_NAME[name] = row
    shas = {}
    for ver in ("v3", "v4"):
        u = _dlower(spec, ver=ver)
        shas[ver] = _DveOpSpec(name=name, opcode=row, uops=u,
                               rd1_en=_dhas_src1(spec)).sha(ver)
    op = _dve_ops.DveOp(name, spec, subdim=False, uops_sha=shas)
    _dve_ops.OPS.append(op)
    _dve_ops.CUSTOM_DVE_SPECS[name] = spec
    return op


FRAC_OP = _make_frac_op()
FRAC_C0 = -0.23549792
FRAC_C1 = 2.0017324

F32 = mybir.dt.float32
F16 = mybir.dt.float16
I32 = mybir.dt.int32
AF = mybir.ActivationFunctionType
OP = mybir.AluOpType

# problem constants
B, S, V, H, L = 2, 2048, 32000, 512, 3
P = 128
KT = H // P            # 4 k-tiles over the H contraction dim
CHUNK = 512            # own tokens per core
HALO = 128             # speculative scan warmup tokens
W = HALO + CHUNK       # 640 window tokens per core
NG = W // P            # 5 embedding gather groups
NCH = [(0, 512), (512, 128)]   # window free-dim chunks (PSUM-bounded)
VC = 500               # vocab chunk for logits
N_CORES = 8
EPS = 1e-5
MAGIC2 = 0x1EF759DF    # rsqrt seed magic, pre-adjusted for hneg=-(var+eps)/2


def build_program(fcwb=9, psgb=4, workb=2, wstb=2):
    nc = bacc.Bacc("TRN2", target_bir_lowering=False, debug=False,
                   enable_asserts=True, num_devices=N_CORES)

    idx_t = nc.dram_tensor("idx", [P, NG], I32, kind="ExternalInput").ap()
    emb_t = nc.dram_tensor("emb", [V, H], F16, kind="ExternalInput").ap()
    wsT_t = nc.dram_tensor("wsT", [L, KT, P, 3 * H], F16, kind="ExternalInput").ap()
    bsg_t = nc.dram_tensor("bsg", [P, L * 12], F32, kind="ExternalInput").ap()
    bshalf_t = nc.dram_tensor("bshalf", [P, L * 4], F32, kind="ExternalInput").ap()
    fcwt_t = nc.dram_tensor("fcwt", [25, P, 10, KT, P], F16, kind="ExternalInput").ap()
    fcb_t = nc.dram_tensor("fcb", [P, V // P], F32, kind="ExternalInput").ap()
    rst_t = nc.dram_tensor("rst", [P, 2], F32, kind="ExternalInput").ap()
    out_t = nc.dram_tensor("out", [V, CHUNK], F16, kind="ExternalOutput").ap()

    with tile.TileContext(nc) as tc:
        with tc.tile_pool(name="singles", bufs=1) as singles, \
             tc.tile_pool(name="persist", bufs=1) as persist, \
             tc.tile_pool(name="fcw", bufs=fcwb) as fcwp:

            # ---- constants / small inputs ----
            idx = singles.tile([P, NG], I32)
            nc.sync.dma_start(out=idx[:], in_=idx_t[:])
            bsg = singles.tile([P, L * 12], F32)
            nc.sync.dma_start(out=bsg[:], in_=bsg_t[:])
            bshalf = singles.tile([P, L * 4], F32)
            nc.sync.dma_start(out=bshalf[:], in_=bshalf_t[:])
            rst = singles.tile([P, 2], F32)
            nc.sync.dma_start(out=rst[:], in_=rst_t[:])
            fcb2 = singles.tile([P, V // P], F32)
            nc.sync.dma_start(out=fcb2[:], in_=fcb_t[:])
            ident16 = singles.tile([P, P], F16)
            make_identity(nc, ident16[:])
            ones16 = singles.tile([P, 1], F16)   # stats-reduce rhs
            nc.vector.memset(ones16[:], 1.0)

            # final activations (channel-major), consumed by phase C
            xf_bf = [persist.tile([P, CHUNK], F16, tag=f"xfbf{k}", name=f"xfbf{k}")
                     for k in range(KT)]

            with tc.tile_pool(name="xpool", bufs=2) as xpool, \
                 tc.tile_pool(name="wst", bufs=wstb) as wstp, \
                 tc.tile_pool(name="work", bufs=workb) as work, \
                 tc.tile_pool(name="scan", bufs=1) as scanp, \
                 tc.tile_pool(name="sq", bufs=1) as sqp, \
                 tc.tile_pool(name="xnp", bufs=1) as xnp, \
                 tc.tile_pool(name="bc", bufs=1) as bcp, \
                 tc.tile_pool(name="stat", bufs=1) as statp, \
                 tc.tile_pool(name="psg", bufs=psgb, space="PSUM") as psg, \
                 tc.tile_pool(name="pss", bufs=1, space="PSUM") as pss, \
                 tc.tile_pool(name="pst", bufs=2, space="PSUM") as pst:

                # ---- phase A: embedding gather + transpose to channel-major
                x = [xpool.tile([P, W], F16, tag=f"x{k}", name=f"xt{k}") for k in range(KT)]
                for g in range(NG):
                    xg = work.tile([P, H], F16, tag="xg")
                    nc.gpsimd.indirect_dma_start(
                        out=xg[:], out_offset=None, in_=emb_t[:],
                        in_offset=IndirectOffsetOnAxis(ap=idx[:, g:g + 1], axis=0),
                    )
                    for k in range(KT):
                        ptr = pst.tile([P, P], F16, tag="pstt", name="ptr")
                        nc.tensor.transpose(
                            out=ptr[:], in_=xg[:, k * P:(k + 1) * P],
                            identity=ident16[:])
                        eng = nc.vector if (g * KT + k) % 2 == 0 else nc.scalar
                        if eng is nc.vector:
                            nc.vector.tensor_copy(
                                out=x[k][:, g * P:(g + 1) * P], in_=ptr[:])
                        else:
                            nc.scalar.copy(
                                out=x[k][:, g * P:(g + 1) * P], in_=ptr[:])

                # ---- helper: transposed LN stats + rsqrt newton ----
                def ln_stats(xs, sqs, g0, ngr, tag):
                    """Per-token -mean*rstd and rstd for token groups
                    [g0, g0+ngr) of the window, returned as a [2*ngr, P] f16
                    SBUF tile: rows [0,ngr) = rstd, rows [ngr, 2*ngr) = mr."""
                    psT = pss.tile([P, 2 * ngr], F32, tag=f"psT{tag}", name="psT")
                    for g in range(ngr):
                        for k in range(KT):
                            nc.tensor.matmul(
                                out=psT[:, g:g + 1],
                                lhsT=xs[k][:, (g0 + g) * P:(g0 + g + 1) * P],
                                rhs=ones16[:],
                                start=(k == 0), stop=(k == KT - 1))
                    for g in range(ngr):
                        for k in range(KT):
                            nc.tensor.matmul(
                                out=psT[:, ngr + g:ngr + g + 1],
                                lhsT=sqs[k][:, (g0 + g) * P:(g0 + g + 1) * P],
                                rhs=ones16[:],
                                start=(k == 0), stop=(k == KT - 1))
                    m2 = statp.tile([P, ngr], F32, tag="m2")
                    hneg = statp.tile([P, ngr], F32, tag="hneg")
                    y = statp.tile([P, ngr], F32, tag="y")
                    t = statp.tile([P, ngr], F32, tag="t")
                    rsmr = statp.tile([P, 2 * ngr], F16, tag="rsmr")
                    # m2 = (sum_x/(H*sqrt2))^2 = m^2/2
                    nc.scalar.activation(out=m2[:], in_=psT[:, 0:ngr],
                                         func=AF.Square,
                                         scale=1.0 / (H * np.sqrt(2.0)))
                    # hneg = m^2/2 - (sum_x2/(2H) + eps/2) = -(var+eps)/2
                    nc.vector.tensor_scalar(
                        out=hneg[:], in0=psT[:, ngr:2 * ngr],
                        scalar1=0.5 / H, scalar2=EPS / 2,
                        op0=OP.mult, op1=OP.add)
                    nc.vector.tensor_sub(hneg[:], m2[:], hneg[:])
                    # rsqrt seed: y = -( (bits(hneg)>>1) - MAGIC2 )
                    nc.vector.tensor_scalar(
                        out=y[:].bitcast(I32), in0=hneg[:].bitcast(I32),
                        scalar1=1, scalar2=None,
                        op0=OP.arith_shift_right)
                    nc.vector.tensor_scalar(
                        out=y[:].bitcast(I32), in0=y[:].bitcast(I32),
                        scalar1=MAGIC2, scalar2=None, op0=OP.subtract)
                    nc.vector.tensor_scalar_mul(y[:].bitcast(I32),
                                                y[:].bitcast(I32), -1)
                    for _ in range(2):
                        nc.vector.tensor_mul(t[:], y[:], y[:])
                        nc.vector.tensor_mul(t[:], t[:], hneg[:])
                        nc.vector.scalar_tensor_tensor(
                            out=y[:], in0=t[:], scalar=1.5, in1=y[:],
                            op0=OP.add, op1=OP.mult)
                    nc.vector.tensor_copy(out=rsmr[:, 0:ngr], in_=y[:])
                    # mr = -(sum_x/H)*rstd
                    nc.vector.scalar_tensor_tensor(
                        out=rsmr[:, ngr:2 * ngr], in0=psT[:, 0:ngr],
                        scalar=-1.0 / H, in1=y[:], op0=OP.mult, op1=OP.mult)
                    # transpose each column separately so every row lands
                    # on partition 0 (partition_broadcast requirement)
                    rows = statp.tile([1, 2 * NG * P], F16, tag="rows",
                                      name="rows")
                    for q in range(2 * ngr):
                        ptrq = pst.tile([1, P], F16, tag="pstt", name="ptrq")
                        nc.tensor.transpose(out=ptrq[:], in_=rsmr[:, q:q + 1],
                                            identity=ident16[:])
                        if q % 2 == 0:
                            nc.vector.tensor_copy(
                                out=rows[0:1, q * P:(q + 1) * P], in_=ptrq[:])
                        else:
                            nc.scalar.copy(
                                out=rows[0:1, q * P:(q + 1) * P], in_=ptrq[:])
                    return rows

                # ---- phase B: L recurrent layers ----
                for l in range(L):
                    wst = wstp.tile([P, KT * 3 * H], F16, tag="wst")
                    for kk in range(KT):
                        nc.sync.dma_start(
                            out=wst[:, kk * 3 * H:(kk + 1) * 3 * H],
                            in_=wsT_t[l, kk])

                    if l == 0:
                        sq = [sqp.tile([P, W], F16, tag=f"sq{k}", name=f"sq{k}")
                              for k in range(KT)]
                        for k in range(KT):
                            nc.gpsimd.tensor_mul(sq[k][:], x[k][:], x[k][:])

                    rows = ln_stats(x, sq, 0, NG, "b")

                    # broadcast rstd/mr across partitions (Pool engine)
                    rb = bcp.tile([P, W], F16, tag="rb")
                    mb = bcp.tile([P, W], F16, tag="mb")
                    for g in range(NG):
                        nc.gpsimd.partition_broadcast(
                            rb[:, g * P:(g + 1) * P],
                            rows[0:1, g * P:(g + 1) * P])
                        nc.gpsimd.partition_broadcast(
                            mb[:, g * P:(g + 1) * P],
                            rows[0:1, (NG + g) * P:(NG + g + 1) * P])

                    # xn = (x - m) * rstd  (affine folded into weights)
                    xn = [xnp.tile([P, W], F16, tag=f"xn{k}", name=f"xn{k}") for k in range(KT)]
                    for k in range(KT):
                        nc.vector.tensor_mul(xn[k][:], x[k][:], rb[:])
                        nc.vector.tensor_add(xn[k][:], xn[k][:], mb[:])

                    # --- gates GEMM + nonlinearities + scan ---
                    sq2 = [None] * KT
                    fp = [scanp.tile([P, W], F16, tag=f"fp{k}", name=f"fp{k}") for k in range(KT)]
                    vv = [scanp.tile([P, W], F16, tag=f"vv{k}", name=f"vv{k}") for k in range(KT)]
                    hh = [scanp.tile([P, W], F16, tag=f"h{k}", name=f"h{k}") for k in range(KT)]
                    x2 = [xpool.tile([P, W], F16, tag=f"x{k}", name=f"xt{k}") for k in range(KT)]
                    for (o, n) in NCH:
                        for k in range(KT):
                            def gate_mm(gate):
                                pg = psg.tile([P, 512], F32, tag="pg")
                                for kk in range(KT):
                                    c0 = kk * 3 * H + gate * H + k * P
                                    nc.tensor.matmul(
                                        out=pg[:, :n],
                                        lhsT=(wst[:, c0:c0 + P]),
                                        rhs=(xn[kk][:, o:o + n]),
                                        start=(kk == 0), stop=(kk == KT - 1))
                                return pg

                            pg_f = gate_mm(0)
                            sf = work.tile([P, 512], F16, tag="sf")
                            nc.scalar.activation(
                                out=sf[:, :n], in_=pg_f[:, :n], func=AF.Sigmoid,
                                bias=bsg[:, l * 12 + k:l * 12 + k + 1])
                            pg_i = gate_mm(1)
                            si = work.tile([P, 512], F16, tag="si")
                            nc.scalar.activation(
                                out=si[:, :n], in_=pg_i[:, :n], func=AF.Sigmoid,
                                bias=bsg[:, l * 12 + 4 + k:l * 12 + 4 + k + 1])
                            nc.vector._custom_dve(
                                FRAC_OP, out=fp[k][:, o:o + n],
                                in0=sf[:, :n], in1=si[:, :n],
                                s0=FRAC_C0, s1=FRAC_C1)
                            pg_t = gate_mm(2)
                            sg = work.tile([P, 512], F16, tag="sg")
                            nc.scalar.activation(
                                out=sg[:, :n], in_=pg_t[:, :n], func=AF.Sigmoid,
                                bias=bsg[:, l * 12 + 8 + k:l * 12 + 8 + k + 1])
                            lin = work.tile([P, 512], F16, tag="lin")
                            nc.scalar.activation(
                                out=lin[:, :n], in_=pg_t[:, :n], func=AF.Identity,
                                bias=bshalf[:, l * 4 + k:l * 4 + k + 1])
                            g16 = work.tile([P, 512], F16, tag="g16")
                            nc.vector.tensor_max(g16[:, :n], lin[:, :n], sg[:, :n])
                            ip16 = work.tile([P, 512], F16, tag="ip16")
                            nc.vector.tensor_scalar(
                                out=ip16[:, :n], in0=fp[k][:, o:o + n],
                                scalar1=-1.0, scalar2=1.0,
                                op0=OP.mult, op1=OP.add)
                            nc.vector.tensor_mul(vv[k][:, o:o + n], ip16[:, :n],
                                                 g16[:, :n])
                            if o == 0:
                                # boundary reset at own-region start (exact
                                # for j==0 cores)
                                t1 = work.tile([P, 1], F32, tag="t1")
                                nc.vector.tensor_mul(
                                    t1[:], fp[k][:, HALO:HALO + 1], rst[:, 1:2])
                                nc.vector.tensor_add(
                                    vv[k][:, HALO:HALO + 1], t1[:],
                                    vv[k][:, HALO:HALO + 1])
                                nc.vector.tensor_mul(
                                    fp[k][:, HALO:HALO + 1],
                                    fp[k][:, HALO:HALO + 1], rst[:, 0:1])
                            else:
                                nc.vector.tensor_tensor_scan(
                                    out=hh[k][:], data0=fp[k][:],
                                    data1=vv[k][:],
                                    initial=0.5, op0=OP.mult, op1=OP.add)
                                nc.vector.tensor_add(x2[k][:], hh[k][:],
                                                     x[k][:])
                                sq2[k] = sqp.tile([P, W], F16, tag=f"sq{k}",
                                                  name=f"sq{k}")
                                nc.gpsimd.tensor_mul(sq2[k][:], x2[k][:],
                                                     x2[k][:])
                    x = x2
                    sq = sq2

                # ---- final LayerNorm (own tokens = groups 1..4) ----
                rows2 = ln_stats(x, sq, 1, NG - 1, "f")
                rb2 = bcp.tile([P, CHUNK], F16, tag="rb2")
                mb2 = bcp.tile([P, CHUNK], F16, tag="mb2")
                for g in range(NG - 1):
                    nc.gpsimd.partition_broadcast(
                        rb2[:, g * P:(g + 1) * P],
                        rows2[0:1, g * P:(g + 1) * P])
                    nc.gpsimd.partition_broadcast(
                        mb2[:, g * P:(g + 1) * P],
                        rows2[0:1, (NG - 1 + g) * P:(NG + g) * P])
                for k in range(KT):
                    nc.vector.tensor_mul(xf_bf[k][:], x[k][:, HALO:], rb2[:])
                    nc.vector.tensor_add(xf_bf[k][:], xf_bf[k][:], mb2[:])

            # ---- phase C: logits GEMM (own 512 tokens x full vocab) ----
            VG = 10   # vocab tiles per fcw load (25 groups of 10)
            with tc.tile_pool(name="osb", bufs=8) as osbp, \
                 tc.tile_pool(name="pso", bufs=8, space="PSUM") as pso:
                for vg in range(25):
                    fcw = fcwp.tile([P, VG, KT, P], F16, tag="fcw")
                    nc.gpsimd.dma_start(out=fcw[:], in_=fcwt_t[vg])
                    for j in range(VG):
                        vt = vg * VG + j
                        po = pso.tile([P, CHUNK], F32, tag="po")
                        for k in range(KT):
                            nc.tensor.matmul(
                                out=po[:], lhsT=fcw[:, j, k, :],
                                rhs=xf_bf[k][:],
                                start=(k == 0), stop=(k == KT - 1))
                        osb = osbp.tile([P, CHUNK], F16, tag="osb")
                        if vt % 2 == 0:
                            nc.scalar.activation(out=osb[:], in_=po[:],
                                                 func=AF.Identity,
                                                 bias=fcb2[:, vt:vt + 1])
                        else:
                            nc.vector.tensor_scalar(
                                out=osb[:], in0=po[:],
                                scalar1=fcb2[:, vt:vt + 1], scalar2=None,
                                op0=OP.add)
                        (nc.sync if vt % 2 == 0 else nc.scalar).dma_start(
                            out=out_t[vt * P:(vt + 1) * P, :], in_=osb[:])

    nc.compile()
    return nc


_CACHED = None


def _get_program():
    global _CACHED
    if _CACHED is None:
        _CACHED = build_program()
    return _CACHED


def prep_inputs(ids, emb, Ws, bs, ln_w, ln_b, fln_w, fc_w, fc_b):
    """Host-side layout prep -> per-core input maps."""
    ids = np.asarray(ids)
    emb = np.asarray(emb, dtype=np.float32)
    Ws = np.asarray(Ws, dtype=np.float32)
    bs = np.asarray(bs, dtype=np.float32)
    ln_w = np.asarray(ln_w, dtype=np.float32)
    ln_b = np.asarray(ln_b, dtype=np.float32)
    fln_w = np.asarray(fln_w, dtype=np.float32)
    fc_w = np.asarray(fc_w, dtype=np.float32)
    fc_b = np.asarray(fc_b, dtype=np.float32)

    emb16 = np.ascontiguousarray(emb).astype(np.float16)

    # fold ln_w into the gate weights, ln_b into the gate biases
    # Ws'[l] = Ws[l] * ln_w[l][None,:]; bias'[l] = bs[l] + Ws[l] @ ln_b[l]
    wsT = np.ascontiguousarray(
        np.stack([(Ws[l] * ln_w[l][None, :]).T.reshape(KT, P, 3 * H)
                  for l in range(L)])).astype(np.float16)
    bias = np.stack([bs[l] + Ws[l] @ ln_b[l] for l in range(L)])  # [L, 3H]

    # per-partition gate biases, grouped [l][gate][k]
    bsg = np.empty((P, L * 12), np.float32)
    bshalf = np.empty((P, L * 4), np.float32)
    for l in range(L):
        for gate in range(3):
            for k in range(KT):
                bsg[:, l * 12 + gate * 4 + k] = \
                    bias[l, gate * H + k * P:gate * H + (k + 1) * P]
        for k in range(KT):
            bshalf[:, l * 4 + k] = bias[l, 2 * H + k * P:2 * H + (k + 1) * P] + 0.5

    # fold fln_w into fc_w; fc_w'.T tiled [25, 128, 10, KT, 128] f16
    fcw = fc_w * fln_w[None, :]
    fcwt = np.ascontiguousarray(
        fcw.T.reshape(KT, P, 25, 10, P).transpose(2, 1, 3, 0, 4)).astype(
            np.float16)
    fcb2 = np.ascontiguousarray(fc_b.reshape(V // P, P).T)

    shared = {"emb": emb16, "wsT": wsT, "bsg": bsg, "bshalf": bshalf,
              "fcwt": fcwt, "fcb": fcb2}

    in_maps = []
    for c in range(N_CORES):
        b, j = divmod(c, 4)
        own0 = j * CHUNK
        win = np.zeros(W, np.int32)
        if j == 0:
            win[HALO:] = ids[b, :CHUNK]
        else:
            win[:] = ids[b, own0 - HALO:own0 + CHUNK]
        idxt = np.ascontiguousarray(win.reshape(NG, P).T)
        rstc = np.empty((P, 2), np.float32)
        rstc[:, 0] = 0.0 if j == 0 else 1.0   # multiplies f at window pos HALO
        rstc[:, 1] = 0.5 if j == 0 else 0.0   # adds f*this to v at pos HALO
        in_maps.append({**shared, "idx": idxt, "rst": rstc})
    return in_maps


def kernel(ids, emb, Ws, bs, ln_w, ln_b, fln_w, fc_w, fc_b):
    nc = _get_program()
    in_maps = prep_inputs(ids, emb, Ws, bs, ln_w, ln_b, fln_w, fc_w, fc_b)
    res = run_bass_kernel_spmd(nc, in_maps, list(range(N_CORES)))
    out = np.empty((B, S, V), np.float32)
    for c in range(N_CORES):
        b, j = divmod(c, 4)
        out[b, j * CHUNK:(j + 1) * CHUNK, :] = \
            res.results[c]["out"].T.astype(np.float32)
    return out


# revision 12
# speedup vs baseline: 1.2760x; 1.0339x over previous
"""Trainium2 Bass kernel for a 3-layer minLSTM-style NLP model.

Model (per reference):
  x = emb[ids]                                   (B,S,E) = (2,2048,512)
  3 x { xn = LN(x); gates = xn @ Ws.T + bs;
        f' = sig(f)/(sig(f)+sig(i)); i' = 1-f';
        v = i' * g(tilde), g(x) = max(x+0.5, sigmoid(x));
        h_t = f'_t h_{t-1} + v_t  (h_0 = 0.5);  x = h + x }
  xf = LN(x) * fln_w;  logits = xf @ fc_w.T + fc_b    (B,S,32000)

Sharding (8 cores, zero collectives):
  core c -> (batch b=c//4, seq chunk j=c%4 of 512 tokens). Each core runs a
  640-token window (128-token halo before its own 512) through the recurrent
  stack; the forget-product decays the unknown initial state to ~0 over the
  halo, and a per-core reset constant makes j==0 exact at the batch start.
  Each core computes logits for its own 512 tokens against the full vocab.

Key layout tricks:
  - LN affine (ln_w/ln_b) folded into the gate weights/biases on host, and
    fln_w folded into fc_w, so the device only applies (x-m)*rstd.
  - Per-token LN stats computed TRANSPOSED (tokens on partitions) via 1-row
    matmuls; rsqrt via magic-constant Newton on tiny [128,5] tiles; result
    transposed back and broadcast across partitions by the Pool engine.
  - Activations f16 everywhere; output logits written f16 and upcast on host.
"""

import sys

if "/opt/trn_rl_repo" not in sys.path:
    sys.path.insert(0, "/opt/trn_rl_repo")

import numpy as np

import concourse.bass as bass
import concourse.bacc as bacc
import concourse.tile as tile
from concourse import mybir
from concourse.bass import IndirectOffsetOnAxis
from concourse.bass_utils import run_bass_kernel_spmd
from concourse.masks import make_identity
from concourse import dve_ops as _dve_ops
from concourse.dve_spec import AluOp as _DAlu, Bin as _DBin, Spec as _DSpec, \
    Src0 as _DS0, Src1 as _DS1, C0 as _DC0, C1 as _DC1, lower as _dlower, \
    _has_src1 as _dhas_src1
from concourse.dve_uop import DveOpSpec as _DveOpSpec


def _make_frac_op():
    """Custom DVE op: out = in0 / (in0 + in1), one bit-trick seed + one
    Newton pass (~1.7e-3 rel err). Fuses the minLSTM gate normalization
    f' = sig(f)/(sig(f)+sig(i)) into a single DVE instruction."""
    name = "FRAC_SIG_FAST_ANT"
    for op in _dve_ops.OPS:
        if op.name == name:
            return op
    _z = _DS0 + _DS1
    _nz = _DBin(_DAlu.BITWISE_NOT, _z, _z)
    _y0 = _nz * _DC0
    _y1 = _y0 * (_DC1 - _z * _y0)

    def _ref(in0, in1, c0, c1, c2):
        z = in0.astype(np.float32) + in1.astype(np.float32)
        nz = (~z.view(np.int32)).view(np.float32)
        y0 = nz * np.float32(c0)
        y1 = (y0 * (np.float32(c1) - z * y0)).astype(np.float32)
        return in0.astype(np.float32) * y1

    spec = _DSpec(body=_DS0 * _y1, reference=_ref)
    row = max(_dve_ops._SUB_OPCODE_FOR_NAME.values()) + 1
    _dve_ops._SUB_OPCODE_FOR_NAME[name] = row
    shas = {}
    for ver in ("v3", "v4"):
        u = _dlower(spec, ver=ver)
        shas[ver] = _DveOpSpec(name=name, opcode=row, uops=u,
                               rd1_en=_dhas_src1(spec)).sha(ver)
    op = _dve_ops.DveOp(name, spec, subdim=False, uops_sha=shas)
    _dve_ops.OPS.append(op)
    _dve_ops.CUSTOM_DVE_SPECS[name] = spec
    return op


FRAC_OP = _make_frac_op()
FRAC_C0 = -0.23549792
FRAC_C1 = 2.0017324

F32 = mybir.dt.float32
F16 = mybir.dt.float16
I32 = mybir.dt.int32
AF = mybir.ActivationFunctionType
OP = mybir.AluOpType

# problem constants
B, S, V, H, L = 2, 2048, 32000, 512, 3
P = 128
KT = H // P            # 4 k-tiles over the H contraction dim
CHUNK = 512            # own tokens per core
HALO = 128             # speculative scan warmup tokens
W = HALO + CHUNK       # 640 window tokens per core
NG = W // P            # 5 embedding gather groups
NCH = [(0, 512), (512, 128)]   # window free-dim chunks (PSUM-bounded)
VC = 500               # vocab chunk for logits
N_CORES = 8
EPS = 1e-5
MAGIC2 = 0x1EF759DF    # rsqrt seed magic, pre-adjusted for hneg=-(var+eps)/2


def build_program(fcwb=9, psgb=4, workb=2, wstb=2):
    nc = bacc.Bacc("TRN2", target_bir_lowering=False, debug=False,
                   enable_asserts=True, num_devices=N_CORES)

    idx_t = nc.dram_tensor("idx", [P, NG], I32, kind="ExternalInput").ap()
    emb_t = nc.dram_tensor("emb", [V, H], F16, kind="ExternalInput").ap()
    wsT_t = nc.dram_tensor("wsT", [L, KT, P, 3 * H], F16, kind="ExternalInput").ap()
    bsg_t = nc.dram_tensor("bsg", [P, L * 12], F32, kind="ExternalInput").ap()
    bshalf_t = nc.dram_tensor("bshalf", [P, L * 4], F32, kind="ExternalInput").ap()
    fcwt_t = nc.dram_tensor("fcwt", [25, P, 10, KT, P], F16, kind="ExternalInput").ap()
    fcb_t = nc.dram_tensor("fcb", [P, V // P], F32, kind="ExternalInput").ap()
    rst_t = nc.dram_tensor("rst", [P, 2], F32, kind="ExternalInput").ap()
    out_t = nc.dram_tensor("out", [V, CHUNK], F16, kind="ExternalOutput").ap()

    with tile.TileContext(nc) as tc:
        with tc.tile_pool(name="singles", bufs=1) as singles, \
             tc.tile_pool(name="persist", bufs=1) as persist, \
             tc.tile_pool(name="fcw", bufs=fcwb) as fcwp:

            # ---- constants / small inputs ----
            idx = singles.tile([P, NG], I32)
            nc.sync.dma_start(out=idx[:], in_=idx_t[:])
            bsg = singles.tile([P, L * 12], F32)
            nc.sync.dma_start(out=bsg[:], in_=bsg_t[:])
            bshalf = singles.tile([P, L * 4], F32)
            nc.sync.dma_start(out=bshalf[:], in_=bshalf_t[:])
            rst = singles.tile([P, 2], F32)
            nc.sync.dma_start(out=rst[:], in_=rst_t[:])
            fcb2 = singles.tile([P, V // P], F32)
            nc.sync.dma_start(out=fcb2[:], in_=fcb_t[:])
            ident16 = singles.tile([P, P], F16)
            make_identity(nc, ident16[:])
            ones16 = singles.tile([P, 1], F16)   # stats-reduce rhs
            nc.vector.memset(ones16[:], 1.0)

            # final activations (channel-major), consumed by phase C
            xf_bf = [persist.tile([P, CHUNK], F16, tag=f"xfbf{k}", name=f"xfbf{k}")
                     for k in range(KT)]

            with tc.tile_pool(name="xpool", bufs=2) as xpool, \
                 tc.tile_pool(name="wst", bufs=wstb) as wstp, \
                 tc.tile_pool(name="work", bufs=workb) as work, \
                 tc.tile_pool(name="scan", bufs=1) as scanp, \
                 tc.tile_pool(name="sq", bufs=1) as sqp, \
                 tc.tile_pool(name="xnp", bufs=1) as xnp, \
                 tc.tile_pool(name="bc", bufs=1) as bcp, \
                 tc.tile_pool(name="stat", bufs=1) as statp, \
                 tc.tile_pool(name="psg", bufs=psgb, space="PSUM") as psg, \
                 tc.tile_pool(name="pss", bufs=1, space="PSUM") as pss, \
                 tc.tile_pool(name="pst", bufs=2, space="PSUM") as pst:

                # ---- phase A: embedding gather + transpose to channel-major
                x = [xpool.tile([P, W], F16, tag=f"x{k}", name=f"xt{k}") for k in range(KT)]
                xgs = []
                for g in range(NG):
                    xg = work.tile([P, H], F16, tag=f"xg{g}", name=f"xg{g}")
                    nc.gpsimd.indirect_dma_start(
                        out=xg[:], out_offset=None, in_=emb_t[:],
                        in_offset=IndirectOffsetOnAxis(ap=idx[:, g:g + 1], axis=0),
                    )
                    xgs.append(xg)
                for g in range(NG):
                    xg = xgs[g]
                    for k in range(KT):
                        ptr = pst.tile([P, P], F16, tag="pstt", name="ptr")
                        nc.tensor.transpose(
                            out=ptr[:], in_=xg[:, k * P:(k + 1) * P],
                            identity=ident16[:])
                        eng = nc.vector if (g * KT + k) % 2 == 0 else nc.scalar
                        if eng is nc.vector:
                            nc.vector.tensor_copy(
                                out=x[k][:, g * P:(g + 1) * P], in_=ptr[:])
                        else:
                            nc.scalar.copy(
                                out=x[k][:, g * P:(g + 1) * P], in_=ptr[:])

                # ---- helper: transposed LN stats + rsqrt newton ----
                def ln_stats(xs, sqs, g0, ngr, tag):
                    """Per-token -mean*rstd and rstd for token groups
                    [g0, g0+ngr) of the window, returned as a [2*ngr, P] f16
                    SBUF tile: rows [0,ngr) = rstd, rows [ngr, 2*ngr) = mr."""
                    psT = pss.tile([P, 2 * ngr], F32, tag=f"psT{tag}", name="psT")
                    for g in range(ngr):
                        for k in range(KT):
                            nc.tensor.matmul(
                                out=psT[:, g:g + 1],
                                lhsT=xs[k][:, (g0 + g) * P:(g0 + g + 1) * P],
                                rhs=ones16[:],
                                start=(k == 0), stop=(k == KT - 1))
                    for g in range(ngr):
                        for k in range(KT):
                            nc.tensor.matmul(
                                out=psT[:, ngr + g:ngr + g + 1],
                                lhsT=sqs[k][:, (g0 + g) * P:(g0 + g + 1) * P],
                                rhs=ones16[:],
                                start=(k == 0), stop=(k == KT - 1))
                    m2 = statp.tile([P, ngr], F32, tag="m2")
                    hneg = statp.tile([P, ngr], F32, tag="hneg")
                    y = statp.tile([P, ngr], F32, tag="y")
                    t = statp.tile([P, ngr], F32, tag="t")
                    rsmr = statp.tile([P, 2 * ngr], F16, tag="rsmr")
                    # m2 = (sum_x/(H*sqrt2))^2 = m^2/2
                    nc.scalar.activation(out=m2[:], in_=psT[:, 0:ngr],
                                         func=AF.Square,
                                         scale=1.0 / (H * np.sqrt(2.0)))
                    # hneg = m^2/2 - (sum_x2/(2H) + eps/2) = -(var+eps)/2
                    nc.vector.tensor_scalar(
                        out=hneg[:], in0=psT[:, ngr:2 * ngr],
                        scalar1=0.5 / H, scalar2=EPS / 2,
                        op0=OP.mult, op1=OP.add)
                    nc.vector.tensor_sub(hneg[:], m2[:], hneg[:])
                    # rsqrt seed: y = -( (bits(hneg)>>1) - MAGIC2 )
                    nc.vector.tensor_scalar(
                        out=y[:].bitcast(I32), in0=hneg[:].bitcast(I32),
                        scalar1=1, scalar2=None,
                        op0=OP.arith_shift_right)
                    nc.vector.tensor_scalar(
                        out=y[:].bitcast(I32), in0=y[:].bitcast(I32),
                        scalar1=MAGIC2, scalar2=None, op0=OP.subtract)
                    nc.vector.tensor_scalar_mul(y[:].bitcast(I32),
                                                y[:].bitcast(I32), -1)
                    for _ in range(2):
                        nc.vector.tensor_mul(t[:], y[:], y[:])
                        nc.vector.tensor_mul(t[:], t[:], hneg[:])
                        nc.vector.scalar_tensor_tensor(
                            out=y[:], in0=t[:], scalar=1.5, in1=y[:],
                            op0=OP.add, op1=OP.mult)
                    nc.vector.tensor_copy(out=rsmr[:, 0:ngr], in_=y[:])
                    # mr = -(sum_x/H)*rstd
                    nc.vector.scalar_tensor_tensor(
                        out=rsmr[:, ngr:2 * ngr], in0=psT[:, 0:ngr],
                        scalar=-1.0 / H, in1=y[:], op0=OP.mult, op1=OP.mult)
                    # transpose each column separately so every row lands
                    # on partition 0 (partition_broadcast requirement)
                    rows = statp.tile([1, 2 * NG * P], F16, tag="rows",
                                      name="rows")
                    for q in range(2 * ngr):
                        ptrq = pst.tile([1, P], F16, tag="pstt", name="ptrq")
                        nc.tensor.transpose(out=ptrq[:], in_=rsmr[:, q:q + 1],
                                            identity=ident16[:])
                        if q % 2 == 0:
                            nc.vector.tensor_copy(
                                out=rows[0:1, q * P:(q + 1) * P], in_=ptrq[:])
                        else:
                            nc.scalar.copy(
                                out=rows[0:1, q * P:(q + 1) * P], in_=ptrq[:])
                    return rows

                # ---- phase B: L recurrent layers ----
                for l in range(L):
                    wst = wstp.tile([P, KT * 3 * H], F16, tag="wst")
                    for kk in range(KT):
                        nc.sync.dma_start(
                            out=wst[:, kk * 3 * H:(kk + 1) * 3 * H],
                            in_=wsT_t[l, kk])

                    if l == 0:
                        sq = [sqp.tile([P, W], F16, tag=f"sq{k}", name=f"sq{k}")
                              for k in range(KT)]
                        for k in range(KT):
                            nc.gpsimd.tensor_mul(sq[k][:], x[k][:], x[k][:])

                    rows = ln_stats(x, sq, 0, NG, "b")

                    # broadcast rstd/mr across partitions (Pool engine)
                    rb = bcp.tile([P, W], F16, tag="rb")
                    mb = bcp.tile([P, W], F16, tag="mb")
                    for g in range(NG):
                        nc.gpsimd.partition_broadcast(
                            rb[:, g * P:(g + 1) * P],
                            rows[0:1, g * P:(g + 1) * P])
                        nc.gpsimd.partition_broadcast(
                            mb[:, g * P:(g + 1) * P],
                            rows[0:1, (NG + g) * P:(NG + g + 1) * P])

                    # xn = (x - m) * rstd  (affine folded into weights)
                    xn = [xnp.tile([P, W], F16, tag=f"xn{k}", name=f"xn{k}") for k in range(KT)]
                    for k in range(KT):
                        nc.vector.tensor_mul(xn[k][:], x[k][:], rb[:])
                        nc.vector.tensor_add(xn[k][:], xn[k][:], mb[:])

                    # --- gates GEMM + nonlinearities + scan ---
                    sq2 = [None] * KT
                    fp = [scanp.tile([P, W], F16, tag=f"fp{k}", name=f"fp{k}") for k in range(KT)]
                    vv = [scanp.tile([P, W], F16, tag=f"vv{k}", name=f"vv{k}") for k in range(KT)]
                    hh = [scanp.tile([P, W], F16, tag=f"h{k}", name=f"h{k}") for k in range(KT)]
                    x2 = [xpool.tile([P, W], F16, tag=f"x{k}", name=f"xt{k}") for k in range(KT)]
                    for (o, n) in NCH:
                        for k in range(KT):
                            def gate_mm(gate):
                                pg = psg.tile([P, 512], F32, tag="pg")
                                for kk in range(KT):
                                    c0 = kk * 3 * H + gate * H + k * P
                                    nc.tensor.matmul(
                                        out=pg[:, :n],
                                        lhsT=(wst[:, c0:c0 + P]),
                                        rhs=(xn[kk][:, o:o + n]),
                                        start=(kk == 0), stop=(kk == KT - 1))
                                return pg

                            pg_f = gate_mm(0)
                            sf = work.tile([P, 512], F16, tag="sf")
                            nc.scalar.activation(
                                out=sf[:, :n], in_=pg_f[:, :n], func=AF.Sigmoid,
                                bias=bsg[:, l * 12 + k:l * 12 + k + 1])
                            pg_i = gate_mm(1)
                            si = work.tile([P, 512], F16, tag="si")
                            nc.scalar.activation(
                                out=si[:, :n], in_=pg_i[:, :n], func=AF.Sigmoid,
                                bias=bsg[:, l * 12 + 4 + k:l * 12 + 4 + k + 1])
                            nc.vector._custom_dve(
                                FRAC_OP, out=fp[k][:, o:o + n],
                                in0=sf[:, :n], in1=si[:, :n],
                                s0=FRAC_C0, s1=FRAC_C1)
                            pg_t = gate_mm(2)
                            sg = work.tile([P, 512], F16, tag="sg")
                            nc.scalar.activation(
                                out=sg[:, :n], in_=pg_t[:, :n], func=AF.Sigmoid,
                                bias=bsg[:, l * 12 + 8 + k:l * 12 + 8 + k + 1])
                            lin = work.tile([P, 512], F16, tag="lin")
                            nc.scalar.activation(
                                out=lin[:, :n], in_=pg_t[:, :n], func=AF.Identity,
                                bias=bshalf[:, l * 4 + k:l * 4 + k + 1])
                            g16 = work.tile([P, 512], F16, tag="g16")
                            nc.vector.tensor_max(g16[:, :n], lin[:, :n], sg[:, :n])
                            ip16 = work.tile([P, 512], F16, tag="ip16")
                            nc.vector.tensor_scalar(
                                out=ip16[:, :n], in0=fp[k][:, o:o + n],
                                scalar1=-1.0, scalar2=1.0,
                                op0=OP.mult, op1=OP.add)
                            nc.vector.tensor_mul(vv[k][:, o:o + n], ip16[:, :n],
                                                 g16[:, :n])
                            if o == 0:
                                # boundary reset at own-region start (exact
                                # for j==0 cores)
                                t1 = work.tile([P, 1], F32, tag="t1")
                                nc.vector.tensor_mul(
                                    t1[:], fp[k][:, HALO:HALO + 1], rst[:, 1:2])
                                nc.vector.tensor_add(
                                    vv[k][:, HALO:HALO + 1], t1[:],
                                    vv[k][:, HALO:HALO + 1])
                                nc.vector.tensor_mul(
                                    fp[k][:, HALO:HALO + 1],
                                    fp[k][:, HALO:HALO + 1], rst[:, 0:1])
                            else:
                                nc.vector.tensor_tensor_scan(
                                    out=hh[k][:], data0=fp[k][:],
                                    data1=vv[k][:],
                                    initial=0.5, op0=OP.mult, op1=OP.add)
                                nc.vector.tensor_add(x2[k][:], hh[k][:],
                                                     x[k][:])
                                sq2[k] = sqp.tile([P, W], F16, tag=f"sq{k}",
                                                  name=f"sq{k}")
                                nc.gpsimd.tensor_mul(sq2[k][:], x2[k][:],
                                                     x2[k][:])
                    x = x2
                    sq = sq2

                # ---- final LayerNorm (own tokens = groups 1..4) ----
                rows2 = ln_stats(x, sq, 1, NG - 1, "f")
                rb2 = bcp.tile([P, CHUNK], F16, tag="rb2")
                mb2 = bcp.tile([P, CHUNK], F16, tag="mb2")
                for g in range(NG - 1):
                    nc.gpsimd.partition_broadcast(
                        rb2[:, g * P:(g + 1) * P],
                        rows2[0:1, g * P:(g + 1) * P])
                    nc.gpsimd.partition_broadcast(
                        mb2[:, g * P:(g + 1) * P],
                        rows2[0:1, (NG - 1 + g) * P:(NG + g) * P])
                for k in range(KT):
                    nc.vector.tensor_mul(xf_bf[k][:], x[k][:, HALO:], rb2[:])
                    nc.vector.tensor_add(xf_bf[k][:], xf_bf[k][:], mb2[:])

            # ---- phase C: logits GEMM (own 512 tokens x full vocab) ----
            VG = 10   # vocab tiles per fcw load (25 groups of 10)
            with tc.tile_pool(name="osb", bufs=8) as osbp, \
                 tc.tile_pool(name="pso", bufs=8, space="PSUM") as pso:
                for vg in range(25):
                    fcw = fcwp.tile([P, VG, KT, P], F16, tag="fcw")
                    nc.gpsimd.dma_start(out=fcw[:], in_=fcwt_t[vg])
                    for j in range(VG):
                        vt = vg * VG + j
                        po = pso.tile([P, CHUNK], F32, tag="po")
                        for k in range(KT):
                            nc.tensor.matmul(
                                out=po[:], lhsT=fcw[:, j, k, :],
                                rhs=xf_bf[k][:],
                                start=(k == 0), stop=(k == KT - 1))
                        osb = osbp.tile([P, CHUNK], F16, tag="osb")
                        if vt % 2 == 0:
                            nc.scalar.activation(out=osb[:], in_=po[:],
                                                 func=AF.Identity,
                                                 bias=fcb2[:, vt:vt + 1])
                        else:
                            nc.vector.tensor_scalar(
                                out=osb[:], in0=po[:],
                                scalar1=fcb2[:, vt:vt + 1], scalar2=None,
                                op0=OP.add)
                        (nc.sync if vt % 2 == 0 else nc.scalar).dma_start(
                            out=out_t[vt * P:(vt + 1) * P, :], in_=osb[:])

    nc.compile()
    return nc


_CACHED = None


def _get_program():
    global _CACHED
    if _CACHED is None:
        _CACHED = build_program()
    return _CACHED


def prep_inputs(ids, emb, Ws, bs, ln_w, ln_b, fln_w, fc_w, fc_b):
    """Host-side layout prep -> per-core input maps."""
    ids = np.asarray(ids)
    emb = np.asarray(emb, dtype=np.float32)
    Ws = np.asarray(Ws, dtype=np.float32)
    bs = np.asarray(bs, dtype=np.float32)
    ln_w = np.asarray(ln_w, dtype=np.float32)
    ln_b = np.asarray(ln_b, dtype=np.float32)
    fln_w = np.asarray(fln_w, dtype=np.float32)
    fc_w = np.asarray(fc_w, dtype=np.float32)
    fc_b = np.asarray(fc_b, dtype=np.float32)

    emb16 = np.ascontiguousarray(emb).astype(np.float16)

    # fold ln_w into the gate weights, ln_b into the gate biases
    # Ws'[l] = Ws[l] * ln_w[l][None,:]; bias'[l] = bs[l] + Ws[l] @ ln_b[l]
    wsT = np.ascontiguousarray(
        np.stack([(Ws[l] * ln_w[l][None, :]).T.reshape(KT, P, 3 * H)
                  for l in range(L)])).astype(np.float16)
    bias = np.stack([bs[l] + Ws[l] @ ln_b[l] for l in range(L)])  # [L, 3H]

    # per-partition gate biases, grouped [l][gate][k]
    bsg = np.empty((P, L * 12), np.float32)
    bshalf = np.empty((P, L * 4), np.float32)
    for l in range(L):
        for gate in range(3):
            for k in range(KT):
                bsg[:, l * 12 + gate * 4 + k] = \
                    bias[l, gate * H + k * P:gate * H + (k + 1) * P]
        for k in range(KT):
            bshalf[:, l * 4 + k] = bias[l, 2 * H + k * P:2 * H + (k + 1) * P] + 0.5

    # fold fln_w into fc_w; fc_w'.T tiled [25, 128, 10, KT, 128] f16
    fcw = fc_w * fln_w[None, :]
    fcwt = np.ascontiguousarray(
        fcw.T.reshape(KT, P, 25, 10, P).transpose(2, 1, 3, 0, 4)).astype(
            np.float16)
    fcb2 = np.ascontiguousarray(fc_b.reshape(V // P, P).T)

    shared = {"emb": emb16, "wsT": wsT, "bsg": bsg, "bshalf": bshalf,
              "fcwt": fcwt, "fcb": fcb2}

    in_maps = []
    for c in range(N_CORES):
        b, j = divmod(c, 4)
        own0 = j * CHUNK
        win = np.zeros(W, np.int32)
        if j == 0:
            win[HALO:] = ids[b, :CHUNK]
        else:
            win[:] = ids[b, own0 - HALO:own0 + CHUNK]
        idxt = np.ascontiguousarray(win.reshape(NG, P).T)
        rstc = np.empty((P, 2), np.float32)
        rstc[:, 0] = 0.0 if j == 0 else 1.0   # multiplies f at window pos HALO
        rstc[:, 1] = 0.5 if j == 0 else 0.0   # adds f*this to v at pos HALO
        in_maps.append({**shared, "idx": idxt, "rst": rstc})
    return in_maps


def kernel(ids, emb, Ws, bs, ln_w, ln_b, fln_w, fc_w, fc_b):
    nc = _get_program()
    in_maps = prep_inputs(ids, emb, Ws, bs, ln_w, ln_b, fln_w, fc_w, fc_b)
    res = run_bass_kernel_spmd(nc, in_maps, list(range(N_CORES)))
    out = np.empty((B, S, V), np.float32)
    for c in range(N_CORES):
        b, j = divmod(c, 4)
        out[b, j * CHUNK:(j + 1) * CHUNK, :] = \
            res.results[c]["out"].T.astype(np.float32)
    return out
